# revision 36
# baseline (speedup 1.0000x reference)
"""Trainium2 Bass kernel for nn_Attention_54778012893268.

Fused QKV projection + RoPE + non-causal SDPA + output projection.
B=4, T=2048, C=2048, H=16, D=128, fp32 in / bf16 partial out.

Sharding: 8 cores = (batch b, head-group hg) pairs; b = core//2, hg = core%2.
Each core handles one batch's tokens and 8 of the 16 heads end-to-end,
producing two partial [T, C] bf16 outputs (heads 0-3 and 4-7 of its
group); the host upcasts and sums the partials across cores.

Design (v8; v7 was 735us, ACT-bound in the attention phase):
- Projection order q -> v -> k, so by ~6% into the k phase head-0's
  q/k/v are all ready and the first two attention pairs (scores + exp +
  AV + denominator tree) are pumped INTO the k-phase window via a
  generator that yields one kt-unit at a time. The ACT exp stream
  (290us total) starts ~100us earlier than v7, so the attention phase
  is PE-bound instead of ACT-bound.
- x is NOT kept resident: a 32-slot ring re-loads it per phase
  (3x8MB of DMA, amortized), freeing 32KB/partition for the early-
  attention pools. All q/v/k weights share one 48-slot ring whose WAR
  dependencies sequence the prefetches naturally.
- every matmul operand is bf16 (PSUM f32); rel err ~9e-3 vs 2e-2 gate.
- softmax denominator: bf16 pairwise DVE tree over the 16 E tiles per
  pair + one ones-matmul per chunk; normalization deferred one pair
  (pend queue) so the PE FIFO never blocks on the DVE tail.
- ynorm is split per (head, T-half) so the final pair's normalization
  only gates the p3 tiles that actually read it; p3 ti 0-7 stream
  while pair-15's tail chain completes, and a reserve of p1 groups
  fills the remaining boundary bubble.
- DMA: all tensors host-pre-tiled to >=1KB rows; loads alternate
  between the two trigger queues (sync/scalar).
"""

import math
import sys
from collections import deque

import numpy as np

sys.path.insert(0, "/opt/trn_rl_repo")

P = 128
T = 2048
C = 2048
HPC = 8          # heads per core
D = 128
CH = 512         # T-chunk (PSUM bank width at fp32)
NCH = T // CH    # 4
KT = C // P      # 16 contraction tiles
TT = T // P      # 16 token tiles
SCALE = 1.0 / math.sqrt(D)
ROPE_BASE = 10000.0

WARMUP_MM = 220  # junk matmuls bridging the initial DMA ramp
P1_RESERVE = 6   # p1 groups kept back to fill the attention->p3 boundary

_CACHED_NC = None


def build_nc():
    import concourse.bass as bass
    import concourse.tile as tile
    from concourse import bacc, mybir

    F32 = mybir.dt.float32
    BF16 = mybir.dt.bfloat16
    ts = bass.ts

    nc = bacc.Bacc("TRN2", target_bir_lowering=False, debug=False, num_devices=8)

    xtp = nc.dram_tensor("xtp", [NCH, KT, P, CH], BF16, kind="ExternalInput").ap()
    wqd = nc.dram_tensor("wqd", [KT, 2, P, CH], BF16, kind="ExternalInput").ap()
    wkd = nc.dram_tensor("wkd", [KT, 2, P, CH], BF16, kind="ExternalInput").ap()
    wvd = nc.dram_tensor("wvd", [KT, 2, P, CH], BF16, kind="ExternalInput").ap()
    wpd = nc.dram_tensor("wpd", [HPC, P, C], BF16, kind="ExternalInput").ap()
    cosm = nc.dram_tensor("cosm", [P, T], BF16, kind="ExternalInput").ap()
    sinm = nc.dram_tensor("sinm", [P, T], BF16, kind="ExternalInput").ap()
    onesd = nc.dram_tensor("onesd", [P, P], BF16, kind="ExternalInput").ap()
    out = nc.dram_tensor("out", [T, C], BF16, kind="ExternalOutput").ap()
    outa = nc.dram_tensor("outa", [T, C], BF16, kind="ExternalOutput").ap()

    # pair-swap shuffle mask (within each 32-partition quadrant)
    SWAP_MASK = [i ^ 1 for i in range(32)]

    with tile.TileContext(nc) as tc:
        from contextlib import ExitStack

        with ExitStack() as outer:
            cpool = outer.enter_context(tc.tile_pool(name="const", bufs=1))
            qkres = outer.enter_context(tc.tile_pool(name="qkres", bufs=1))
            vres = outer.enter_context(
                tc.tile_pool(name="vres", bufs=1, side="right"))

            ones = cpool.tile([P, P], BF16, tag="ones")
            nc.sync.dma_start(ones[:], onesd)

            # ---------------- projection scaffolding ----------------
            es1 = ExitStack()
            mpool = es1.enter_context(tc.tile_pool(name="masks", bufs=2))
            rp = es1.enter_context(tc.tile_pool(name="rope", bufs=2))
            wpool = es1.enter_context(tc.tile_pool(name="w", bufs=3 * KT))
            xpool = es1.enter_context(tc.tile_pool(name="xch", bufs=2 * KT))

            def load_masks(ci):
                cs = mpool.tile([P, CH], BF16, tag="cos", name=f"cos{ci}")
                nc.scalar.dma_start(cs[:], cosm[:, ts(ci, CH)])
                sn = mpool.tile([P, CH], BF16, tag="sin", name=f"sin{ci}")
                nc.sync.dma_start(sn[:], sinm[:, ts(ci, CH)])
                return cs, sn

            # x ring: each load_x re-requests a chunk; the 32-slot ring's
            # WAR deps serialize against the previous consumer phase.
            dma_engs = None  # set after nc engines exist; 3 trigger queues

            def load_x(ci):
                tiles = []
                for kt in range(KT):
                    xtl = xpool.tile([P, CH], BF16, tag="x")
                    dma_engs[kt % 2].dma_start(xtl[:], xtp[ci, kt])
                    tiles.append(xtl)
                return tiles

            # weight ring: q-h0, q-h1, v-h0 | v-h1, k-h0, k-h1 (wraps)
            def load_w(wt, w_dram, half):
                for kt in range(KT):
                    w0 = wpool.tile([P, CH], BF16, tag="w")
                    dma_engs[(kt + 1) % 2].dma_start(w0[:], w_dram[kt, half])
                    wt[half][kt] = w0

            wq_t = {0: {}, 1: {}}
            wv_t = {0: {}, 1: {}}
            wk_t = {0: {}, 1: {}}

            # ---- ramp: interleave weight + x loads across both queues ----
            dma_engs = (nc.sync, nc.scalar)
            load_w(wq_t, wqd, 0)
            msk = {0: load_masks(0)}
            xq = {0: load_x(0)}
            load_w(wq_t, wqd, 1)
            msk[1] = load_masks(1)
            xq[1] = load_x(1)
            load_w(wv_t, wvd, 0)   # fresh slots 32-47, no WAR

            ps_q = tc.alloc_tile_pool(name="psq", bufs=4, space="PSUM")

            # warm the PE HAM across the DMA ramp; the junk exp preloads
            # the ACT exp table before the attention phase
            warm_ps = ps_q.tile([P, 64], F32, tag="mm", name="warmps")
            for wi in range(WARMUP_MM):
                nc.tensor.matmul(warm_ps[:], ones[:], ones[:, :64],
                                 start=(wi == 0), stop=(wi == WARMUP_MM - 1))
            wexp = rp.tile([P, CH], BF16, tag="r0", name="warmexp")
            nc.scalar.activation(wexp[:, :64], warm_ps[:],
                                 mybir.ActivationFunctionType.Exp, scale=SCALE)

            q_t = {}   # (h, ci) -> [128 d, 512 t] bf16
            k_t = {}
            v_t = {}   # (vc, ti) -> [128 t, 512 f] bf16

            def rope(ps, tag, cs, sn):
                e0 = rp.tile([P, CH], BF16, tag="r0")
                nc.vector.tensor_copy(e0[:], ps[:])
                e1 = rp.tile([P, CH], BF16, tag="r1")
                nc.vector.stream_shuffle(e1[:], e0[:], SWAP_MASK)
                a = rp.tile([P, CH], BF16, tag="r1", name="ra")
                nc.vector.tensor_mul(a[:], e0[:], cs[:])
                b = rp.tile([P, CH], BF16, tag="r0", name="rb")
                nc.vector.tensor_mul(b[:], e1[:], sn[:])
                ro = qkres.tile([P, CH], BF16, tag=tag)
                nc.vector.tensor_add(ro[:], a[:], b[:])
                return ro

            # ---------------- Phase Q ----------------
            for ci in range(NCH):
                cs, sn = msk[ci]
                for fi in range(HPC):
                    ps = ps_q.tile([P, CH], F32, tag="mm")
                    for kt in range(KT):
                        nc.tensor.matmul(
                            ps[:],
                            wq_t[fi // 4][kt][:, ts(fi % 4, P)],
                            xq[ci][kt][:],
                            start=(kt == 0),
                            stop=(kt == KT - 1),
                        )
                    q_t[fi, ci] = rope(ps, f"q{fi}_{ci}", cs, sn)
                # schedule re-loads as ring slots free up
                if ci == 0:
                    msk[2] = load_masks(2)
                    xq[2] = load_x(2)
                elif ci == 1:
                    msk[3] = load_masks(3)
                    xq[3] = load_x(3)
                elif ci == 2:
                    xq[0] = load_x(0)       # for v phase
                    load_w(wv_t, wvd, 1)    # into q-h0 slots
                elif ci == 3:
                    xq[1] = load_x(1)
            ps_q.release()

            # ---------------- Phase V ----------------
            ps_v = tc.alloc_tile_pool(name="psv", bufs=4, space="PSUM")
            load_w(wk_t, wkd, 0)            # into q-h1 slots
            msk[0] = load_masks(0)          # masks for the k phase
            msk[1] = load_masks(1)
            for ci in range(NCH):
                for vc in range(2):
                    for sub in range(4):
                        ti = 4 * ci + sub
                        ps = ps_v.tile([P, CH], F32, tag="mmv")
                        for kt in range(KT):
                            nc.tensor.matmul(
                                ps[:],
                                xq[ci][kt][:, ts(sub, P)],
                                wv_t[vc][kt][:],
                                start=(kt == 0),
                                stop=(kt == KT - 1),
                            )
                        sb = vres.tile([P, CH], BF16, tag=f"v{vc}_{ti}")
                        nc.vector.tensor_copy(sb[:], ps[:])
                        v_t[vc, ti] = sb
                if ci == 0:
                    xq[2] = load_x(2)
                elif ci == 1:
                    xq[3] = load_x(3)
                elif ci == 2:
                    xq[0] = load_x(0)       # for k phase
                elif ci == 3:
                    xq[1] = load_x(1)
            ps_v.release()
            load_w(wk_t, wkd, 1)            # into v-h0 slots (free at v end)

            # ------------- attention pools (open before k) -------------
            ep = outer.enter_context(
                tc.tile_pool(name="ee", bufs=4, side="right"))
            spool = outer.enter_context(
                tc.tile_pool(name="st", bufs=5, side="right"))
            ycp = outer.enter_context(
                tc.tile_pool(name="yc", bufs=4, side="right"))

            psS = tc.alloc_tile_pool(name="psS", bufs=2, space="PSUM")
            psY = tc.alloc_tile_pool(name="psY", bufs=2, space="PSUM")
            ps_k = tc.alloc_tile_pool(name="psk", bufs=2, space="PSUM")

            # ---------------- attention machinery ----------------
            # cp-major order within each head quad: out-projection tranches
            # unlock progressively (p1 ti0-7 after pair 3, p1 ti8-15 after
            # pair 7, p3 ti0-7 after pair 11), giving the ACT-deficit pairs
            # PE filler work throughout.
            pairs = ([(h, 0) for h in range(4)] + [(h, 1) for h in range(4)]
                     + [(h, 0) for h in range(4, 8)]
                     + [(h, 1) for h in range(4, 8)])
            NP = len(pairs)
            s_store = {pi: {} for pi in range(NP)}
            pend = deque()
            psd_pool = [None]   # psD opens after ps_k closes
            ynorm = {}          # (h, half) -> [128 d, 1024 t] bf16
            wpt = []
            rcp_pool = [None]
            pap_pool = [None]

            def s_mm(pi, kt):
                h, cp = pairs[pi]
                sp = psS.tile([P, 2 * CH], F32, tag="s", name=f"s{kt}")
                kT = k_t[h, kt // 4][:, ts(kt % 4, P)]
                nc.tensor.matmul(sp[:, 0:CH], kT, q_t[h, 2 * cp][:],
                                 start=True, stop=True)
                nc.tensor.matmul(sp[:, CH:2 * CH], kT, q_t[h, 2 * cp + 1][:],
                                 start=True, stop=True)
                return sp

            tails_done = [0]

            def emit_tail(t):
                h_, cp_, sfin, yc0, yc1 = t
                psD = psd_pool[0]
                rcp = rcp_pool[0]
                yn = ynorm[h_, cp_]
                d0 = psD.tile([P, CH], F32, tag="d", name="d0")
                nc.tensor.matmul(d0[:], ones[:], sfin[:, 0:CH],
                                 start=True, stop=True)
                d1 = psD.tile([P, CH], F32, tag="d", name="d1")
                nc.tensor.matmul(d1[:], ones[:], sfin[:, CH:2 * CH],
                                 start=True, stop=True)
                r0 = rcp.tile([P, CH], F32, tag="rc")
                nc.vector.reciprocal_approx_fast(r0[:], d0[:])
                nc.vector.tensor_mul(yn[:, 0:CH], yc0[:], r0[:])
                r1 = rcp.tile([P, CH], F32, tag="rc")
                nc.vector.reciprocal_approx_fast(r1[:], d1[:])
                nc.vector.tensor_mul(yn[:, CH:2 * CH], yc1[:], r1[:])
                tails_done[0] += 1

            # out-projection filler: partial A (heads 0-3 -> outa) and
            # partial B (heads 4-7 -> out), streamed into the attention
            # phase's ACT-deficit PE slots as their ynorm tranches unlock.
            fillq = ([("p1", ti, oc) for ti in range(TT) for oc in range(NCH)]
                     + [("p3", ti, oc) for ti in range(8) for oc in range(NCH)])
            p3q = [("p3", ti, oc) for ti in range(8, TT) for oc in range(NCH)]

            def fill_ready(ent):
                kind, ti, oc = ent
                need = (4 if ti < 8 else 8) if kind == "p1" else \
                    (12 if ti < 8 else 16)
                return tails_done[0] >= need

            def emit_fill(pool=None):
                if not fillq:
                    if not p3q:
                        return False
                    ent = p3q.pop(0)
                elif fill_ready(fillq[0]):
                    ent = fillq.pop(0)
                else:
                    return False
                kind, ti, oc = ent
                ps = (pool or psd_pool[0]).tile([P, CH], F32, tag="d",
                                                name="pf")
                h0_ = 0 if kind == "p1" else 4
                for hh in range(h0_, h0_ + 4):
                    nc.tensor.matmul(
                        ps[:],
                        ynorm[hh, ti // 8][:, ts(ti % 8, P)],
                        wpt[hh][:, ts(oc, CH)],
                        start=(hh == h0_),
                        stop=(hh == h0_ + 3),
                    )
                ob = pap_pool[0].tile([P, CH], BF16, tag="pa")
                nc.vector.tensor_copy(ob[:], ps[:])
                dst = outa if kind == "p1" else out
                (nc.sync if oc % 2 == 0 else nc.scalar).dma_start(
                    dst[ts(ti, P), ts(oc, CH)], ob[:])
                return True

            def pair_gen(pi):
                h, cp = pairs[pi]
                vc, vo = h // 4, (h % 4) * P
                y0 = psY.tile([P, CH], F32, tag="y", name="y0")
                y1 = psY.tile([P, CH], F32, tag="y", name="y1")
                s_tiles = s_store[pi]
                es = {}
                lvl = {}
                for kt in range(TT):
                    if kt == 3 and pend and psd_pool[0] is not None:
                        emit_tail(pend.popleft())
                    if kt == 9 and len(pend) > 1 and psd_pool[0] is not None:
                        emit_tail(pend.popleft())
                    if (kt % 2 == 1 and pi >= 4 and
                            len(fillq) > P1_RESERVE):
                        emit_fill()
                    e = ep.tile([P, 2 * CH], BF16, tag="e")
                    nc.scalar.activation(
                        e[:], s_tiles.pop(kt)[:],
                        mybir.ActivationFunctionType.Exp, scale=SCALE,
                    )
                    es[kt] = e
                    if kt + 2 < TT:
                        s_tiles[kt + 2] = s_mm(pi, kt + 2)
                    elif pi + 1 < NP:
                        # emit the NEXT pair's first score groups early so
                        # the tail (av15 waiting on exp15) never blocks them
                        s_store[pi + 1][kt + 2 - TT] = s_mm(pi + 1, kt + 2 - TT)
                    elif len(fillq) > P1_RESERVE:
                        emit_fill()
                    vT = v_t[vc, kt][:, vo:vo + P]
                    nc.tensor.matmul(y0[:], vT, e[:, 0:CH],
                                     start=(kt == 0), stop=(kt == TT - 1))
                    nc.tensor.matmul(y1[:], vT, e[:, CH:2 * CH],
                                     start=(kt == 0), stop=(kt == TT - 1))
                    # denominator tree: bf16 pairwise adds on DVE
                    if kt % 2 == 1:
                        t1 = spool.tile([P, 2 * CH], BF16, tag="t")
                        nc.vector.tensor_add(t1[:], es.pop(kt - 1)[:],
                                             es.pop(kt)[:])
                        lvl[1, kt // 2] = t1
                    for L in (1, 2, 3):
                        j = (kt + 1) // (1 << (L + 1))
                        if (kt + 1) % (1 << (L + 1)) == 0:
                            t2 = spool.tile([P, 2 * CH], BF16, tag="t")
                            nc.vector.tensor_add(
                                t2[:], lvl.pop((L, 2 * j - 2))[:],
                                lvl.pop((L, 2 * j - 1))[:])
                            lvl[L + 1, j - 1] = t2
                    yield
                sfin = lvl.pop((4, 0))
                # free the y psum banks early so the next pair's AV
                # accumulation never waits on this pair's recip/mul
                yc0 = ycp.tile([P, CH], BF16, tag="yc", name="yc0")
                nc.vector.tensor_copy(yc0[:], y0[:])
                yc1 = ycp.tile([P, CH], BF16, tag="yc", name="yc1")
                nc.vector.tensor_copy(yc1[:], y1[:])
                pend.append((h, cp, sfin, yc0, yc1))

            def all_pairs():
                for pi in range(NP):
                    yield from pair_gen(pi)

            pump = all_pairs()
            pumped = [0]

            def pump_units(n):
                for _ in range(n):
                    if next(pump, StopIteration) is StopIteration:
                        return False
                    pumped[0] += 1
                return True

            # ---------------- Phase K (+ injected pairs 0-1) ----------------
            started = [False]
            for ci in range(NCH):
                cs, sn = msk[ci]
                for fi in range(HPC):
                    ps = ps_k.tile([P, CH], F32, tag="mmk")
                    for kt in range(KT):
                        nc.tensor.matmul(
                            ps[:],
                            wk_t[fi // 4][kt][:, ts(fi % 4, P)],
                            xq[ci][kt][:],
                            start=(kt == 0),
                            stop=(kt == KT - 1),
                        )
                    k_t[fi, ci] = rope(ps, f"k{fi}_{ci}", cs, sn)
                    # pump early-pair units once their k chunks exist:
                    # pair-0 unit kt needs k_t[0, (kt+2)//4] for its s_mm
                    if fi >= 1:
                        if not started[0] and ci == 0:
                            s_store[0][0] = s_mm(0, 0)
                            s_store[0][1] = s_mm(0, 1)
                            started[0] = True
                        # pair-0 unit kt emits s_mm(kt+2) needing k chunk
                        # (kt+2)//4; pair 1 (h1) consumes the same chunks.
                        # Cap at 24 so pair-1's tail lands in attention.
                        max_units = (4 * (ci + 1) - 2) if ci < 3 else 24
                        per_win = 2 if ci < 3 else 3
                        pump_units(min(per_win, max_units - pumped[0]))
                if ci == 0:
                    msk[2] = load_masks(2)
                    xq[2] = load_x(2)
                elif ci == 1:
                    msk[3] = load_masks(3)
                    xq[3] = load_x(3)
            ps_k.release()
            es1.close()

            # ---------------- attention (remaining pairs) ----------------
            ynp = outer.enter_context(tc.tile_pool(name="ynorm", bufs=1))
            wpp = outer.enter_context(tc.tile_pool(name="wp", bufs=1))
            rcp = outer.enter_context(
                tc.tile_pool(name="rc", bufs=2, side="right"))
            pap = outer.enter_context(
                tc.tile_pool(name="pa", bufs=4, side="right"))
            psD = tc.alloc_tile_pool(name="psD", bufs=2, space="PSUM")
            psd_pool[0] = psD
            rcp_pool[0] = rcp
            pap_pool[0] = pap
            for h in range(HPC):
                for half in range(2):
                    ynorm[h, half] = ynp.tile(
                        [P, 2 * CH], BF16, tag=f"yn{h}_{half}",
                        name=f"ynorm{h}_{half}")
            for h in range(HPC):
                wtl = wpp.tile([P, C], BF16, tag=f"wp{h}", name=f"wpt{h}")
                (nc.sync if h % 2 == 0 else nc.scalar).dma_start(
                    wtl[:], wpd[h])
                wpt.append(wtl)

            while pump_units(16):
                pass

            # ---------------- Phase 3: boundary + remaining out-proj ----
            # flush the last tails with the reserved fill groups covering
            # the recip/mul chain, then hand the freed PSUM banks to the
            # final p3 tranche (ti 8-15, which needs the last tail).
            emit_fill()
            emit_fill()
            while pend:
                emit_tail(pend.popleft())
            while fillq:
                emit_fill()
            psD.release()
            psY.release()
            psS.release()
            ps3 = tc.alloc_tile_pool(name="ps3", bufs=4, space="PSUM")
            while emit_fill(pool=ps3):
                pass
            ps3.release()

    nc.compile()
    return nc


def get_nc():
    global _CACHED_NC
    if _CACHED_NC is None:
        _CACHED_NC = build_nc()
    return _CACHED_NC


def make_rope_masks():
    half = D // 2
    inv = 1.0 / (ROPE_BASE ** (np.arange(half, dtype=np.float64) * 2.0 / D))
    ang = np.arange(T, dtype=np.float64)[:, None] * inv[None, :]  # [T, half]
    cos = np.cos(ang).T.astype(np.float32)  # [half, T]
    sin = np.sin(ang).T.astype(np.float32)
    cosm = np.empty((P, T), np.float32)
    sinm = np.empty((P, T), np.float32)
    cosm[0::2] = cos
    cosm[1::2] = cos
    sinm[0::2] = -sin
    sinm[1::2] = sin
    return cosm, sinm


def make_in_maps(x, w_attn, w_proj):
    import ml_dtypes
    BF = ml_dtypes.bfloat16

    x = np.asarray(x, dtype=np.float32)
    w_attn = np.asarray(w_attn, dtype=np.float32)
    w_proj = np.asarray(w_proj, dtype=np.float32)
    cosm, sinm = make_rope_masks()
    cosm16 = cosm.astype(BF)
    sinm16 = sinm.astype(BF)
    ones16 = np.ones((P, P), BF)
    in_maps = []
    for core in range(8):
        b, hg = core // 2, core % 2
        h0 = hg * HPC
        rq = slice(h0 * D, (h0 + HPC) * D)
        rk = slice(C + h0 * D, C + (h0 + HPC) * D)
        rv = slice(2 * C + h0 * D, 2 * C + (h0 + HPC) * D)
        # x tiles: [NCH, KT, P, CH] from x[b].T
        xt = np.ascontiguousarray(x[b].T.astype(BF))
        xtp = np.ascontiguousarray(
            xt.reshape(KT, P, NCH, CH).transpose(2, 0, 1, 3))
        # wq/wk/wv: [C, HPC*D] -> [KT, 2, P, CH] (1KB-row half tiles)
        def wtile(w):
            wT = w.T.astype(BF)  # [C, HPC*D]
            return np.ascontiguousarray(
                wT.reshape(KT, P, 2, CH).transpose(0, 2, 1, 3))
        wvd = wtile(w_attn[rv])
        wpT = np.ascontiguousarray(
            w_proj[:, h0 * D:(h0 + HPC) * D].T.astype(BF)).reshape(HPC, P, C)
        in_maps.append({
            "xtp": xtp,
            "wqd": wtile(w_attn[rq]),
            "wkd": wtile(w_attn[rk]),
            "wvd": wvd,
            "wpd": wpT,
            "cosm": cosm16,
            "sinm": sinm16,
            "onesd": ones16,
        })
    return in_maps


def combine_outputs(results):
    B = 4
    out = np.empty((B, T, C), np.float32)
    for b in range(B):
        out[b] = (results[2 * b]["out"].astype(np.float32)
                  + results[2 * b]["outa"].astype(np.float32)
                  + results[2 * b + 1]["out"].astype(np.float32)
                  + results[2 * b + 1]["outa"].astype(np.float32))
    return out


def kernel(x, w_attn, w_proj):
    from concourse.bass_utils import run_bass_kernel_spmd

    nc = get_nc()
    in_maps = make_in_maps(x, w_attn, w_proj)
    res = run_bass_kernel_spmd(nc, in_maps, list(range(8)))
    return combine_outputs(res.results)


# revision 42
# speedup vs baseline: 1.1944x; 1.1944x over previous
"""Trainium2 Bass kernel for nn_Attention_54778012893268.

Fused QKV projection + RoPE + non-causal SDPA + output projection.
B=4, T=2048, C=2048, H=16, D=128, fp32 in / bf16 partial out.

Sharding: 8 cores = (batch b, head-group hg) pairs; b = core//2, hg = core%2.
Each core handles one batch's tokens and 8 of the 16 heads end-to-end,
producing two partial [T, C] bf16 outputs (heads 0-3 and 4-7 of its
group); the host upcasts and sums the partials across cores.

Design (v8; v7 was 735us, ACT-bound in the attention phase):
- Projection order q -> v -> k, so by ~6% into the k phase head-0's
  q/k/v are all ready and the first two attention pairs (scores + exp +
  AV + denominator tree) are pumped INTO the k-phase window via a
  generator that yields one kt-unit at a time. The ACT exp stream
  (290us total) starts ~100us earlier than v7, so the attention phase
  is PE-bound instead of ACT-bound.
- x is NOT kept resident: a 32-slot ring re-loads it per phase
  (3x8MB of DMA, amortized), freeing 32KB/partition for the early-
  attention pools. All q/v/k weights share one 48-slot ring whose WAR
  dependencies sequence the prefetches naturally.
- every matmul operand is bf16 (PSUM f32); rel err ~9e-3 vs 2e-2 gate.
- softmax denominator: bf16 pairwise DVE tree over the 16 E tiles per
  pair + one ones-matmul per chunk; normalization deferred one pair
  (pend queue) so the PE FIFO never blocks on the DVE tail.
- ynorm is split per (head, T-half) so the final pair's normalization
  only gates the p3 tiles that actually read it; p3 ti 0-7 stream
  while pair-15's tail chain completes, and a reserve of p1 groups
  fills the remaining boundary bubble.
- DMA: all tensors host-pre-tiled to >=1KB rows; loads alternate
  between the two trigger queues (sync/scalar).
"""

import math
import sys
from collections import deque

import numpy as np

sys.path.insert(0, "/opt/trn_rl_repo")

P = 128
T = 2048
C = 2048
HPC = 8          # heads per core
D = 128
CH = 512         # T-chunk (PSUM bank width at fp32)
NCH = T // CH    # 4
KT = C // P      # 16 contraction tiles
TT = T // P      # 16 token tiles
SCALE = 1.0 / math.sqrt(D)
ROPE_BASE = 10000.0

WARMUP_MM = 220  # junk matmuls bridging the initial DMA ramp
P1_RESERVE = 6   # p1 groups kept back to fill the attention->p3 boundary

_CACHED_NC = None


def build_nc():
    import concourse.bass as bass
    import concourse.tile as tile
    from concourse import bacc, mybir

    F32 = mybir.dt.float32
    BF16 = mybir.dt.bfloat16
    ts = bass.ts

    nc = bacc.Bacc("TRN2", target_bir_lowering=False, debug=False, num_devices=8)

    xtp = nc.dram_tensor("xtp", [NCH, KT, P, CH], BF16, kind="ExternalInput").ap()
    wqd = nc.dram_tensor("wqd", [KT, 2, P, CH], BF16, kind="ExternalInput").ap()
    wkd = nc.dram_tensor("wkd", [KT, 2, P, CH], BF16, kind="ExternalInput").ap()
    wvd = nc.dram_tensor("wvd", [KT, 2, P, CH], BF16, kind="ExternalInput").ap()
    wpd = nc.dram_tensor("wpd", [HPC, P, C], BF16, kind="ExternalInput").ap()
    cosm = nc.dram_tensor("cosm", [P, T], BF16, kind="ExternalInput").ap()
    sinm = nc.dram_tensor("sinm", [P, T], BF16, kind="ExternalInput").ap()
    onesd = nc.dram_tensor("onesd", [P, P], BF16, kind="ExternalInput").ap()
    out = nc.dram_tensor("out", [T, C], BF16, kind="ExternalOutput").ap()
    outa = nc.dram_tensor("outa", [T, C], BF16, kind="ExternalOutput").ap()

    # pair-swap shuffle mask (within each 32-partition quadrant)
    SWAP_MASK = [i ^ 1 for i in range(32)]

    with tile.TileContext(nc) as tc:
        from contextlib import ExitStack

        with ExitStack() as outer:
            cpool = outer.enter_context(tc.tile_pool(name="const", bufs=1))
            qkres = outer.enter_context(tc.tile_pool(name="qkres", bufs=1))
            vres = outer.enter_context(
                tc.tile_pool(name="vres", bufs=1, side="right"))

            ones = cpool.tile([P, P], BF16, tag="ones")
            nc.sync.dma_start(ones[:], onesd)

            # ---------------- projection scaffolding ----------------
            es1 = ExitStack()
            mpool = es1.enter_context(tc.tile_pool(name="masks", bufs=2))
            rp = es1.enter_context(tc.tile_pool(name="rope", bufs=2))
            wpool = es1.enter_context(tc.tile_pool(name="w", bufs=3 * KT))
            xpool = es1.enter_context(tc.tile_pool(name="xch", bufs=2 * KT))

            def load_masks(ci, sync_only=False):
                eng = nc.sync if sync_only else nc.scalar
                cs = mpool.tile([P, CH], BF16, tag="cos", name=f"cos{ci}")
                eng.dma_start(cs[:], cosm[:, ts(ci, CH)])
                sn = mpool.tile([P, CH], BF16, tag="sin", name=f"sin{ci}")
                nc.sync.dma_start(sn[:], sinm[:, ts(ci, CH)])
                return cs, sn

            # x ring: each load_x re-requests a chunk; the 32-slot ring's
            # WAR deps serialize against the previous consumer phase.
            dma_engs = None  # set after nc engines exist

            def load_x(ci, sync_only=False):
                tiles = []
                for kt in range(KT):
                    xtl = xpool.tile([P, CH], BF16, tag="x")
                    eng = dma_engs[0] if sync_only else dma_engs[kt % 2]
                    eng.dma_start(xtl[:], xtp[ci, kt])
                    tiles.append(xtl)
                return tiles

            # weight ring: q-h0, q-h1, v-h0 | v-h1, k-h0, k-h1 (wraps)
            def load_w(wt, w_dram, half):
                for kt in range(KT):
                    w0 = wpool.tile([P, CH], BF16, tag="w")
                    dma_engs[(kt + 1) % 2].dma_start(w0[:], w_dram[kt, half])
                    wt[half][kt] = w0

            wq_t = {0: {}, 1: {}}
            wv_t = {0: {}, 1: {}}
            wk_t = {0: {}, 1: {}}

            # ---- ramp: interleave weight + x loads across both queues ----
            dma_engs = (nc.sync, nc.scalar)
            load_w(wq_t, wqd, 0)
            msk = {0: load_masks(0)}
            xq = {0: load_x(0)}
            load_w(wq_t, wqd, 1)
            msk[1] = load_masks(1)
            xq[1] = load_x(1)
            load_w(wv_t, wvd, 0)   # fresh slots 32-47, no WAR

            ps_q = tc.alloc_tile_pool(name="psq", bufs=4, space="PSUM")

            # warm the PE HAM across the DMA ramp; the junk exp preloads
            # the ACT exp table before the attention phase
            warm_ps = ps_q.tile([P, 64], F32, tag="mm", name="warmps")
            for wi in range(WARMUP_MM):
                nc.tensor.matmul(warm_ps[:], ones[:], ones[:, :64],
                                 start=(wi == 0), stop=(wi == WARMUP_MM - 1))
            wexp = rp.tile([P, CH], BF16, tag="r0", name="warmexp")
            nc.scalar.activation(wexp[:, :64], warm_ps[:],
                                 mybir.ActivationFunctionType.Exp, scale=SCALE)

            q_t = {}   # (h, ci) -> [128 d, 512 t] bf16
            k_t = {}
            v_t = {}   # (vc, ti) -> [128 t, 512 f] bf16

            def rope(ps, tag, cs, sn):
                e0 = rp.tile([P, CH], BF16, tag="r0")
                nc.vector.tensor_copy(e0[:], ps[:])
                e1 = rp.tile([P, CH], BF16, tag="r1")
                nc.vector.stream_shuffle(e1[:], e0[:], SWAP_MASK)
                a = rp.tile([P, CH], BF16, tag="r1", name="ra")
                nc.vector.tensor_mul(a[:], e0[:], cs[:])
                b = rp.tile([P, CH], BF16, tag="r0", name="rb")
                nc.vector.tensor_mul(b[:], e1[:], sn[:])
                ro = qkres.tile([P, CH], BF16, tag=tag)
                nc.vector.tensor_add(ro[:], a[:], b[:])
                return ro

            # ---------------- Phase Q ----------------
            for ci in range(NCH):
                cs, sn = msk[ci]
                for fi in range(HPC):
                    ps = ps_q.tile([P, CH], F32, tag="mm")
                    for kt in range(KT):
                        nc.tensor.matmul(
                            ps[:],
                            wq_t[fi // 4][kt][:, ts(fi % 4, P)],
                            xq[ci][kt][:],
                            start=(kt == 0),
                            stop=(kt == KT - 1),
                        )
                    q_t[fi, ci] = rope(ps, f"q{fi}_{ci}", cs, sn)
                # schedule re-loads as ring slots free up
                if ci == 0:
                    msk[2] = load_masks(2)
                    xq[2] = load_x(2)
                elif ci == 1:
                    msk[3] = load_masks(3)
                    xq[3] = load_x(3)
                elif ci == 2:
                    xq[0] = load_x(0)       # for v phase
                    load_w(wv_t, wvd, 1)    # into q-h0 slots
                elif ci == 3:
                    xq[1] = load_x(1)
            ps_q.release()

            # ---------------- Phase V ----------------
            ps_v = tc.alloc_tile_pool(name="psv", bufs=4, space="PSUM")
            load_w(wk_t, wkd, 0)            # into q-h1 slots
            msk[0] = load_masks(0)          # masks for the k phase
            msk[1] = load_masks(1)
            for ci in range(NCH):
                for vc in range(2):
                    for sub in range(4):
                        ti = 4 * ci + sub
                        ps = ps_v.tile([P, CH], F32, tag="mmv")
                        for kt in range(KT):
                            nc.tensor.matmul(
                                ps[:],
                                xq[ci][kt][:, ts(sub, P)],
                                wv_t[vc][kt][:],
                                start=(kt == 0),
                                stop=(kt == KT - 1),
                            )
                        sb = vres.tile([P, CH], BF16, tag=f"v{vc}_{ti}")
                        nc.vector.tensor_copy(sb[:], ps[:])
                        v_t[vc, ti] = sb
                if ci == 0:
                    xq[2] = load_x(2)
                elif ci == 1:
                    xq[3] = load_x(3)
                elif ci == 2:
                    xq[0] = load_x(0)       # for k phase
                elif ci == 3:
                    xq[1] = load_x(1)
            ps_v.release()
            load_w(wk_t, wkd, 1)            # into v-h0 slots (free at v end)

            # ------------- attention pools (open before k) -------------
            ep = outer.enter_context(
                tc.tile_pool(name="ee", bufs=4, side="right"))
            spool = outer.enter_context(
                tc.tile_pool(name="st", bufs=5, side="right"))
            ycp = outer.enter_context(
                tc.tile_pool(name="yc", bufs=4, side="right"))

            psS = tc.alloc_tile_pool(name="psS", bufs=2, space="PSUM")
            psY = tc.alloc_tile_pool(name="psY", bufs=2, space="PSUM")
            ps_k = tc.alloc_tile_pool(name="psk", bufs=2, space="PSUM")

            # ---------------- attention machinery ----------------
            # cp-major order within each head quad: out-projection tranches
            # unlock progressively (p1 ti0-7 after pair 3, p1 ti8-15 after
            # pair 7, p3 ti0-7 after pair 11), giving the ACT-deficit pairs
            # PE filler work throughout.
            pairs = ([(h, 0) for h in range(4)] + [(h, 1) for h in range(4)]
                     + [(h, 0) for h in range(4, 8)]
                     + [(h, 1) for h in range(4, 8)])
            NP = len(pairs)
            s_store = {pi: {} for pi in range(NP)}
            pend = deque()
            psd_pool = [None]   # psD opens after ps_k closes
            ynorm = {}          # (h, half) -> [128 d, 1024 t] bf16
            wpt = []
            rcp_pool = [None]
            pap_pool = [None]

            def s_mm(pi, kt):
                h, cp = pairs[pi]
                sp = psS.tile([P, 2 * CH], F32, tag="s", name=f"s{kt}")
                kT = k_t[h, kt // 4][:, ts(kt % 4, P)]
                nc.tensor.matmul(sp[:, 0:CH], kT, q_t[h, 2 * cp][:],
                                 start=True, stop=True)
                nc.tensor.matmul(sp[:, CH:2 * CH], kT, q_t[h, 2 * cp + 1][:],
                                 start=True, stop=True)
                return sp

            tails_done = [0]

            def emit_tail(t):
                h_, cp_, sfin, yc0, yc1 = t
                psD = psd_pool[0]
                rcp = rcp_pool[0]
                yn = ynorm[h_, cp_]
                d0 = psD.tile([P, CH], F32, tag="d", name="d0")
                nc.tensor.matmul(d0[:], ones[:], sfin[:, 0:CH],
                                 start=True, stop=True)
                d1 = psD.tile([P, CH], F32, tag="d", name="d1")
                nc.tensor.matmul(d1[:], ones[:], sfin[:, CH:2 * CH],
                                 start=True, stop=True)
                r0 = rcp.tile([P, CH], F32, tag="rc")
                nc.vector.reciprocal_approx_fast(r0[:], d0[:])
                nc.vector.tensor_mul(yn[:, 0:CH], yc0[:], r0[:])
                r1 = rcp.tile([P, CH], F32, tag="rc")
                nc.vector.reciprocal_approx_fast(r1[:], d1[:])
                nc.vector.tensor_mul(yn[:, CH:2 * CH], yc1[:], r1[:])
                tails_done[0] += 1

            # out-projection filler: partial A (heads 0-3 -> outa) and
            # partial B (heads 4-7 -> out), streamed into the attention
            # phase's ACT-deficit PE slots as their ynorm tranches unlock.
            fillq = ([("p1", ti, oc) for ti in range(TT) for oc in range(NCH)]
                     + [("p3", ti, oc) for ti in range(8) for oc in range(NCH)])
            p3q = [("p3", ti, oc) for ti in range(8, TT) for oc in range(NCH)]

            def fill_ready(ent):
                kind, ti, oc = ent
                need = (4 if ti < 8 else 8) if kind == "p1" else \
                    (12 if ti < 8 else 16)
                return tails_done[0] >= need

            def emit_fill(pool=None):
                if not fillq:
                    if not p3q:
                        return False
                    ent = p3q.pop(0)
                elif fill_ready(fillq[0]):
                    ent = fillq.pop(0)
                else:
                    return False
                kind, ti, oc = ent
                ps = (pool or psd_pool[0]).tile([P, CH], F32, tag="d",
                                                name="pf")
                h0_ = 0 if kind == "p1" else 4
                for hh in range(h0_, h0_ + 4):
                    nc.tensor.matmul(
                        ps[:],
                        ynorm[hh, ti // 8][:, ts(ti % 8, P)],
                        wpt[hh][:, ts(oc, CH)],
                        start=(hh == h0_),
                        stop=(hh == h0_ + 3),
                    )
                ob = pap_pool[0].tile([P, CH], BF16, tag="pa")
                nc.vector.tensor_copy(ob[:], ps[:])
                dst = outa if kind == "p1" else out
                nc.sync.dma_start(dst[ts(ti, P), ts(oc, CH)], ob[:])
                return True

            def pair_gen(pi):
                h, cp = pairs[pi]
                vc, vo = h // 4, (h % 4) * P
                y0 = psY.tile([P, CH], F32, tag="y", name="y0")
                y1 = psY.tile([P, CH], F32, tag="y", name="y1")
                s_tiles = s_store[pi]
                es = {}
                lvl = {}
                for kt in range(TT):
                    if kt == 3 and pend and psd_pool[0] is not None:
                        emit_tail(pend.popleft())
                    if kt == 9 and len(pend) > 1 and psd_pool[0] is not None:
                        emit_tail(pend.popleft())
                    if (kt % 2 == 1 and pi >= 4 and
                            len(fillq) > P1_RESERVE):
                        emit_fill()
                    e = ep.tile([P, 2 * CH], BF16, tag="e")
                    nc.scalar.activation(
                        e[:], s_tiles.pop(kt)[:],
                        mybir.ActivationFunctionType.Exp, scale=SCALE,
                    )
                    es[kt] = e
                    if kt + 2 < TT:
                        s_tiles[kt + 2] = s_mm(pi, kt + 2)
                    elif pi + 1 < NP:
                        # emit the NEXT pair's first score groups early so
                        # the tail (av15 waiting on exp15) never blocks them
                        s_store[pi + 1][kt + 2 - TT] = s_mm(pi + 1, kt + 2 - TT)
                    elif len(fillq) > P1_RESERVE:
                        emit_fill()
                    vT = v_t[vc, kt][:, vo:vo + P]
                    nc.tensor.matmul(y0[:], vT, e[:, 0:CH],
                                     start=(kt == 0), stop=(kt == TT - 1))
                    nc.tensor.matmul(y1[:], vT, e[:, CH:2 * CH],
                                     start=(kt == 0), stop=(kt == TT - 1))
                    # denominator tree: bf16 pairwise adds on DVE
                    if kt % 2 == 1:
                        t1 = spool.tile([P, 2 * CH], BF16, tag="t")
                        nc.vector.tensor_add(t1[:], es.pop(kt - 1)[:],
                                             es.pop(kt)[:])
                        lvl[1, kt // 2] = t1
                    for L in (1, 2, 3):
                        j = (kt + 1) // (1 << (L + 1))
                        if (kt + 1) % (1 << (L + 1)) == 0:
                            t2 = spool.tile([P, 2 * CH], BF16, tag="t")
                            nc.vector.tensor_add(
                                t2[:], lvl.pop((L, 2 * j - 2))[:],
                                lvl.pop((L, 2 * j - 1))[:])
                            lvl[L + 1, j - 1] = t2
                    yield
                sfin = lvl.pop((4, 0))
                # free the y psum banks early so the next pair's AV
                # accumulation never waits on this pair's recip/mul
                yc0 = ycp.tile([P, CH], BF16, tag="yc", name="yc0")
                nc.vector.tensor_copy(yc0[:], y0[:])
                yc1 = ycp.tile([P, CH], BF16, tag="yc", name="yc1")
                nc.vector.tensor_copy(yc1[:], y1[:])
                pend.append((h, cp, sfin, yc0, yc1))

            def all_pairs():
                for pi in range(NP):
                    yield from pair_gen(pi)

            pump = all_pairs()
            pumped = [0]

            def pump_units(n):
                for _ in range(n):
                    if next(pump, StopIteration) is StopIteration:
                        return False
                    pumped[0] += 1
                return True

            # ---------------- Phase K (+ injected pairs 0-1) ----------------
            # From here on the Scalar queue belongs to the exp stream:
            # every DMA trigger costs ~0.65us of issuing-engine time and a
            # WAR-waiting trigger blocks the whole queue, so all further
            # loads/stores trigger from Sync only.
            started = [False]
            for ci in range(NCH):
                if ci == 1:
                    msk[2] = load_masks(2, sync_only=True)
                    xq[2] = load_x(2, sync_only=True)
                elif ci == 2:
                    msk[3] = load_masks(3, sync_only=True)
                    xq[3] = load_x(3, sync_only=True)
                cs, sn = msk[ci]
                for fi in range(HPC):
                    ps = ps_k.tile([P, CH], F32, tag="mmk")
                    for kt in range(KT):
                        nc.tensor.matmul(
                            ps[:],
                            wk_t[fi // 4][kt][:, ts(fi % 4, P)],
                            xq[ci][kt][:],
                            start=(kt == 0),
                            stop=(kt == KT - 1),
                        )
                    k_t[fi, ci] = rope(ps, f"k{fi}_{ci}", cs, sn)
                    # pump early-pair units once their k chunks exist:
                    # pair-0 unit kt needs k_t[0, (kt+2)//4] for its s_mm
                    if fi >= 1:
                        if not started[0] and ci == 0:
                            s_store[0][0] = s_mm(0, 0)
                            s_store[0][1] = s_mm(0, 1)
                            started[0] = True
                        # pair-0 unit kt emits s_mm(kt+2) needing k chunk
                        # (kt+2)//4; pair 1 (h1) consumes the same chunks.
                        # Cap at 24 so pair-1's tail lands in attention.
                        max_units = (4 * (ci + 1) - 2) if ci < 3 else 24
                        per_win = 2 if ci < 3 else 3
                        pump_units(min(per_win, max_units - pumped[0]))
            ps_k.release()
            es1.close()

            # ---------------- attention (remaining pairs) ----------------
            ynp = outer.enter_context(tc.tile_pool(name="ynorm", bufs=1))
            wpp = outer.enter_context(tc.tile_pool(name="wp", bufs=1))
            rcp = outer.enter_context(
                tc.tile_pool(name="rc", bufs=2, side="right"))
            pap = outer.enter_context(
                tc.tile_pool(name="pa", bufs=4, side="right"))
            psD = tc.alloc_tile_pool(name="psD", bufs=2, space="PSUM")
            psd_pool[0] = psD
            rcp_pool[0] = rcp
            pap_pool[0] = pap
            for h in range(HPC):
                for half in range(2):
                    ynorm[h, half] = ynp.tile(
                        [P, 2 * CH], BF16, tag=f"yn{h}_{half}",
                        name=f"ynorm{h}_{half}")
            for h in range(HPC):
                wtl = wpp.tile([P, C], BF16, tag=f"wp{h}", name=f"wpt{h}")
                nc.sync.dma_start(wtl[:], wpd[h])
                wpt.append(wtl)

            while pump_units(16):
                pass

            # ---------------- Phase 3: boundary + remaining out-proj ----
            # flush the last tails with the reserved fill groups covering
            # the recip/mul chain, then hand the freed PSUM banks to the
            # final p3 tranche (ti 8-15, which needs the last tail).
            emit_fill()
            emit_fill()
            while pend:
                emit_tail(pend.popleft())
            while fillq:
                emit_fill()
            psD.release()
            psY.release()
            psS.release()
            ps3 = tc.alloc_tile_pool(name="ps3", bufs=4, space="PSUM")
            while emit_fill(pool=ps3):
                pass
            ps3.release()

    nc.compile()
    return nc


def get_nc():
    global _CACHED_NC
    if _CACHED_NC is None:
        _CACHED_NC = build_nc()
    return _CACHED_NC


def make_rope_masks():
    half = D // 2
    inv = 1.0 / (ROPE_BASE ** (np.arange(half, dtype=np.float64) * 2.0 / D))
    ang = np.arange(T, dtype=np.float64)[:, None] * inv[None, :]  # [T, half]
    cos = np.cos(ang).T.astype(np.float32)  # [half, T]
    sin = np.sin(ang).T.astype(np.float32)
    cosm = np.empty((P, T), np.float32)
    sinm = np.empty((P, T), np.float32)
    cosm[0::2] = cos
    cosm[1::2] = cos
    sinm[0::2] = -sin
    sinm[1::2] = sin
    return cosm, sinm


def make_in_maps(x, w_attn, w_proj):
    import ml_dtypes
    BF = ml_dtypes.bfloat16

    x = np.asarray(x, dtype=np.float32)
    w_attn = np.asarray(w_attn, dtype=np.float32)
    w_proj = np.asarray(w_proj, dtype=np.float32)
    cosm, sinm = make_rope_masks()
    cosm16 = cosm.astype(BF)
    sinm16 = sinm.astype(BF)
    ones16 = np.ones((P, P), BF)
    in_maps = []
    for core in range(8):
        b, hg = core // 2, core % 2
        h0 = hg * HPC
        rq = slice(h0 * D, (h0 + HPC) * D)
        rk = slice(C + h0 * D, C + (h0 + HPC) * D)
        rv = slice(2 * C + h0 * D, 2 * C + (h0 + HPC) * D)
        # x tiles: [NCH, KT, P, CH] from x[b].T
        xt = np.ascontiguousarray(x[b].T.astype(BF))
        xtp = np.ascontiguousarray(
            xt.reshape(KT, P, NCH, CH).transpose(2, 0, 1, 3))
        # wq/wk/wv: [C, HPC*D] -> [KT, 2, P, CH] (1KB-row half tiles)
        def wtile(w):
            wT = w.T.astype(BF)  # [C, HPC*D]
            return np.ascontiguousarray(
                wT.reshape(KT, P, 2, CH).transpose(0, 2, 1, 3))
        wvd = wtile(w_attn[rv])
        wpT = np.ascontiguousarray(
            w_proj[:, h0 * D:(h0 + HPC) * D].T.astype(BF)).reshape(HPC, P, C)
        in_maps.append({
            "xtp": xtp,
            "wqd": wtile(w_attn[rq]),
            "wkd": wtile(w_attn[rk]),
            "wvd": wvd,
            "wpd": wpT,
            "cosm": cosm16,
            "sinm": sinm16,
            "onesd": ones16,
        })
    return in_maps


def combine_outputs(results):
    B = 4
    out = np.empty((B, T, C), np.float32)
    for b in range(B):
        out[b] = (results[2 * b]["out"].astype(np.float32)
                  + results[2 * b]["outa"].astype(np.float32)
                  + results[2 * b + 1]["out"].astype(np.float32)
                  + results[2 * b + 1]["outa"].astype(np.float32))
    return out


def kernel(x, w_attn, w_proj):
    from concourse.bass_utils import run_bass_kernel_spmd

    nc = get_nc()
    in_maps = make_in_maps(x, w_attn, w_proj)
    res = run_bass_kernel_spmd(nc, in_maps, list(range(8)))
    return combine_outputs(res.results)


# revision 48
# speedup vs baseline: 1.2426x; 1.0403x over previous
"""Trainium2 Bass kernel for nn_Attention_54778012893268.

Fused QKV projection + RoPE + non-causal SDPA + output projection.
B=4, T=2048, C=2048, H=16, D=128, fp32 in / bf16 partial out.

Sharding: 8 cores = (batch b, head-group hg) pairs; b = core//2, hg = core%2.
Each core handles one batch's tokens and 8 of the 16 heads end-to-end,
producing two partial [T, C] bf16 outputs (heads 0-3 and 4-7 of its
group); the host upcasts and sums the partials across cores.

Design (v8; v7 was 735us, ACT-bound in the attention phase):
- Projection order q -> v -> k, so by ~6% into the k phase head-0's
  q/k/v are all ready and the first two attention pairs (scores + exp +
  AV + denominator tree) are pumped INTO the k-phase window via a
  generator that yields one kt-unit at a time. The ACT exp stream
  (290us total) starts ~100us earlier than v7, so the attention phase
  is PE-bound instead of ACT-bound.
- x is NOT kept resident: a 32-slot ring re-loads it per phase
  (3x8MB of DMA, amortized), freeing 32KB/partition for the early-
  attention pools. All q/v/k weights share one 48-slot ring whose WAR
  dependencies sequence the prefetches naturally.
- every matmul operand is bf16 (PSUM f32); rel err ~9e-3 vs 2e-2 gate.
- softmax denominator: bf16 pairwise DVE tree over the 16 E tiles per
  pair + one ones-matmul per chunk; normalization deferred one pair
  (pend queue) so the PE FIFO never blocks on the DVE tail.
- ynorm is split per (head, T-half) so the final pair's normalization
  only gates the p3 tiles that actually read it; p3 ti 0-7 stream
  while pair-15's tail chain completes, and a reserve of p1 groups
  fills the remaining boundary bubble.
- DMA: all tensors host-pre-tiled to >=1KB rows; loads alternate
  between the two trigger queues (sync/scalar).
"""

import math
import sys
from collections import deque

import numpy as np

sys.path.insert(0, "/opt/trn_rl_repo")

P = 128
T = 2048
C = 2048
HPC = 8          # heads per core
D = 128
CH = 512         # T-chunk (PSUM bank width at fp32)
NCH = T // CH    # 4
KT = C // P      # 16 contraction tiles
TT = T // P      # 16 token tiles
SCALE = 1.0 / math.sqrt(D)
ROPE_BASE = 10000.0

WARMUP_MM = 270  # junk matmuls bridging the initial DMA ramp
P1_RESERVE = 4   # fill groups kept back to cover the attention->p3 boundary

_CACHED_NC = None


def build_nc():
    import concourse.bass as bass
    import concourse.tile as tile
    from concourse import bacc, mybir

    F32 = mybir.dt.float32
    BF16 = mybir.dt.bfloat16
    ts = bass.ts

    nc = bacc.Bacc("TRN2", target_bir_lowering=False, debug=False, num_devices=8)

    xtp = nc.dram_tensor("xtp", [NCH, KT, P, CH], BF16, kind="ExternalInput").ap()
    wqd = nc.dram_tensor("wqd", [KT, 2, P, CH], BF16, kind="ExternalInput").ap()
    wkd = nc.dram_tensor("wkd", [KT, 2, P, CH], BF16, kind="ExternalInput").ap()
    wvd = nc.dram_tensor("wvd", [KT, 2, P, CH], BF16, kind="ExternalInput").ap()
    wpd = nc.dram_tensor("wpd", [HPC, P, C], BF16, kind="ExternalInput").ap()
    cosm = nc.dram_tensor("cosm", [P, T], BF16, kind="ExternalInput").ap()
    sinm = nc.dram_tensor("sinm", [P, T], BF16, kind="ExternalInput").ap()
    onesd = nc.dram_tensor("onesd", [P, P], BF16, kind="ExternalInput").ap()
    out = nc.dram_tensor("out", [T, C], BF16, kind="ExternalOutput").ap()
    outa = nc.dram_tensor("outa", [T, C], BF16, kind="ExternalOutput").ap()

    # pair-swap shuffle mask (within each 32-partition quadrant)
    SWAP_MASK = [i ^ 1 for i in range(32)]

    with tile.TileContext(nc) as tc:
        from contextlib import ExitStack

        with ExitStack() as outer:
            cpool = outer.enter_context(tc.tile_pool(name="const", bufs=1))
            qkres = outer.enter_context(tc.tile_pool(name="qkres", bufs=1))
            vres = outer.enter_context(
                tc.tile_pool(name="vres", bufs=1, side="right"))

            ones = cpool.tile([P, P], BF16, tag="ones")
            nc.sync.dma_start(ones[:], onesd)

            # ---------------- projection scaffolding ----------------
            es1 = ExitStack()
            mpool = es1.enter_context(tc.tile_pool(name="masks", bufs=2))
            rp = es1.enter_context(tc.tile_pool(name="rope", bufs=2))
            wpool = es1.enter_context(tc.tile_pool(name="w", bufs=3 * KT))
            xpool = es1.enter_context(tc.tile_pool(name="xch", bufs=2 * KT))

            def load_masks(ci, sync_only=False):
                eng = nc.sync if sync_only else nc.scalar
                cs = mpool.tile([P, CH], BF16, tag="cos", name=f"cos{ci}")
                eng.dma_start(cs[:], cosm[:, ts(ci, CH)])
                sn = mpool.tile([P, CH], BF16, tag="sin", name=f"sin{ci}")
                nc.sync.dma_start(sn[:], sinm[:, ts(ci, CH)])
                return cs, sn

            # x ring: each load_x re-requests a chunk; the 32-slot ring's
            # WAR deps serialize against the previous consumer phase.
            dma_engs = None  # set after nc engines exist

            def load_x(ci, sync_only=False):
                tiles = []
                for kt in range(KT):
                    xtl = xpool.tile([P, CH], BF16, tag="x")
                    eng = dma_engs[0] if sync_only else dma_engs[kt % 2]
                    eng.dma_start(xtl[:], xtp[ci, kt])
                    tiles.append(xtl)
                return tiles

            # weight ring: q-h0, q-h1, v-h0 | v-h1, k-h0, k-h1 (wraps)
            def load_w(wt, w_dram, half):
                for kt in range(KT):
                    w0 = wpool.tile([P, CH], BF16, tag="w")
                    dma_engs[(kt + 1) % 2].dma_start(w0[:], w_dram[kt, half])
                    wt[half][kt] = w0

            wq_t = {0: {}, 1: {}}
            wv_t = {0: {}, 1: {}}
            wk_t = {0: {}, 1: {}}

            # ---- ramp: interleave weight + x loads across both queues ----
            dma_engs = (nc.sync, nc.scalar)
            load_w(wq_t, wqd, 0)
            msk = {0: load_masks(0)}
            xq = {0: load_x(0)}
            load_w(wq_t, wqd, 1)
            msk[1] = load_masks(1)
            xq[1] = load_x(1)
            load_w(wv_t, wvd, 0)   # fresh slots 32-47, no WAR

            ps_q = tc.alloc_tile_pool(name="psq", bufs=4, space="PSUM")

            # warm the PE HAM across the DMA ramp; the junk exp preloads
            # the ACT exp table before the attention phase
            warm_ps = ps_q.tile([P, 64], F32, tag="mm", name="warmps")
            for wi in range(WARMUP_MM):
                nc.tensor.matmul(warm_ps[:], ones[:], ones[:, :64],
                                 start=(wi == 0), stop=(wi == WARMUP_MM - 1))
            wexp = rp.tile([P, CH], BF16, tag="r0", name="warmexp")
            nc.scalar.activation(wexp[:, :64], warm_ps[:],
                                 mybir.ActivationFunctionType.Exp, scale=SCALE)

            q_t = {}   # (h, ci) -> [128 d, 512 t] bf16
            k_t = {}
            v_t = {}   # (vc, ti) -> [128 t, 512 f] bf16

            def rope(ps, tag, cs, sn):
                e0 = rp.tile([P, CH], BF16, tag="r0")
                nc.vector.tensor_copy(e0[:], ps[:])
                e1 = rp.tile([P, CH], BF16, tag="r1")
                nc.vector.stream_shuffle(e1[:], e0[:], SWAP_MASK)
                a = rp.tile([P, CH], BF16, tag="r1", name="ra")
                nc.vector.tensor_mul(a[:], e0[:], cs[:])
                b = rp.tile([P, CH], BF16, tag="r0", name="rb")
                nc.vector.tensor_mul(b[:], e1[:], sn[:])
                ro = qkres.tile([P, CH], BF16, tag=tag)
                nc.vector.tensor_add(ro[:], a[:], b[:])
                return ro

            # ---------------- Phase Q ----------------
            for ci in range(NCH):
                cs, sn = msk[ci]
                for fi in range(HPC):
                    ps = ps_q.tile([P, CH], F32, tag="mm")
                    for kt in range(KT):
                        nc.tensor.matmul(
                            ps[:],
                            wq_t[fi // 4][kt][:, ts(fi % 4, P)],
                            xq[ci][kt][:],
                            start=(kt == 0),
                            stop=(kt == KT - 1),
                        )
                    q_t[fi, ci] = rope(ps, f"q{fi}_{ci}", cs, sn)
                # schedule re-loads as ring slots free up
                if ci == 0:
                    msk[2] = load_masks(2)
                    xq[2] = load_x(2)
                elif ci == 1:
                    msk[3] = load_masks(3)
                    xq[3] = load_x(3)
                elif ci == 2:
                    xq[0] = load_x(0)       # for v phase
                    load_w(wv_t, wvd, 1)    # into q-h0 slots
                elif ci == 3:
                    xq[1] = load_x(1)
            ps_q.release()

            # ---------------- Phase V ----------------
            ps_v = tc.alloc_tile_pool(name="psv", bufs=4, space="PSUM")
            load_w(wk_t, wkd, 0)            # into q-h1 slots
            msk[0] = load_masks(0)          # masks for the k phase
            msk[1] = load_masks(1)
            for ci in range(NCH):
                for vc in range(2):
                    for sub in range(4):
                        ti = 4 * ci + sub
                        ps = ps_v.tile([P, CH], F32, tag="mmv")
                        for kt in range(KT):
                            nc.tensor.matmul(
                                ps[:],
                                xq[ci][kt][:, ts(sub, P)],
                                wv_t[vc][kt][:],
                                start=(kt == 0),
                                stop=(kt == KT - 1),
                            )
                        sb = vres.tile([P, CH], BF16, tag=f"v{vc}_{ti}")
                        nc.vector.tensor_copy(sb[:], ps[:])
                        v_t[vc, ti] = sb
                if ci == 0:
                    xq[2] = load_x(2)
                elif ci == 1:
                    xq[3] = load_x(3)
                elif ci == 2:
                    xq[0] = load_x(0)       # for k phase
                elif ci == 3:
                    xq[1] = load_x(1)
            ps_v.release()
            load_w(wk_t, wkd, 1)            # into v-h0 slots (free at v end)

            # ------------- attention pools (open before k) -------------
            ep = outer.enter_context(
                tc.tile_pool(name="ee", bufs=4, side="right"))
            spool = outer.enter_context(
                tc.tile_pool(name="st", bufs=5, side="right"))
            ycp = outer.enter_context(
                tc.tile_pool(name="yc", bufs=4, side="right"))

            psS = tc.alloc_tile_pool(name="psS", bufs=2, space="PSUM")
            psY = tc.alloc_tile_pool(name="psY", bufs=2, space="PSUM")
            ps_k = tc.alloc_tile_pool(name="psk", bufs=2, space="PSUM")

            # ---------------- attention machinery ----------------
            # cp-major order within each head quad: out-projection tranches
            # unlock progressively (p1 ti0-7 after pair 3, p1 ti8-15 after
            # pair 7, p3 ti0-7 after pair 11), giving the ACT-deficit pairs
            # PE filler work throughout.
            pairs = ([(h, 0) for h in range(4)] + [(h, 1) for h in range(4)]
                     + [(h, 0) for h in range(4, 8)]
                     + [(h, 1) for h in range(4, 8)])
            NP = len(pairs)
            s_store = {pi: {} for pi in range(NP)}
            pend = deque()
            psd_pool = [None]   # psD opens after ps_k closes
            ep2_pool = [None]   # wider E ring, opens after k frees SBUF
            ynorm = {}          # (h, half) -> [128 d, 1024 t] bf16
            wpt = []
            rcp_pool = [None]
            pap_pool = [None]

            def s_mm(pi, kt):
                h, cp = pairs[pi]
                sp = psS.tile([P, 2 * CH], F32, tag="s", name=f"s{kt}")
                kT = k_t[h, kt // 4][:, ts(kt % 4, P)]
                nc.tensor.matmul(sp[:, 0:CH], kT, q_t[h, 2 * cp][:],
                                 start=True, stop=True)
                nc.tensor.matmul(sp[:, CH:2 * CH], kT, q_t[h, 2 * cp + 1][:],
                                 start=True, stop=True)
                return sp

            tails_done = [0]

            def emit_tail(t):
                h_, cp_, sfin, yc0, yc1 = t
                psD = psd_pool[0]
                rcp = rcp_pool[0]
                yn = ynorm[h_, cp_]
                d0 = psD.tile([P, CH], F32, tag="d", name="d0")
                nc.tensor.matmul(d0[:], ones[:], sfin[:, 0:CH],
                                 start=True, stop=True)
                d1 = psD.tile([P, CH], F32, tag="d", name="d1")
                nc.tensor.matmul(d1[:], ones[:], sfin[:, CH:2 * CH],
                                 start=True, stop=True)
                r0 = rcp.tile([P, CH], F32, tag="rc")
                nc.vector.reciprocal_approx_fast(r0[:], d0[:])
                nc.vector.tensor_mul(yn[:, 0:CH], yc0[:], r0[:])
                r1 = rcp.tile([P, CH], F32, tag="rc")
                nc.vector.reciprocal_approx_fast(r1[:], d1[:])
                nc.vector.tensor_mul(yn[:, CH:2 * CH], yc1[:], r1[:])
                tails_done[0] += 1

            # out-projection filler: partial A (heads 0-3 -> outa) and
            # partial B (heads 4-7 -> out), streamed into the attention
            # phase's ACT-deficit PE slots as their ynorm tranches unlock.
            fillq = ([("p1", ti, oc) for ti in range(TT) for oc in range(NCH)]
                     + [("p3", ti, oc) for ti in range(8) for oc in range(NCH)])
            p3q = [("p3", ti, oc) for ti in range(8, TT) for oc in range(NCH)]

            def fill_ready(ent):
                kind, ti, oc = ent
                need = (4 if ti < 8 else 8) if kind == "p1" else \
                    (12 if ti < 8 else 16)
                return tails_done[0] >= need

            def emit_fill(pool=None):
                if not fillq:
                    if not p3q:
                        return False
                    ent = p3q.pop(0)
                elif fill_ready(fillq[0]):
                    ent = fillq.pop(0)
                else:
                    return False
                kind, ti, oc = ent
                ps = (pool or psd_pool[0]).tile([P, CH], F32, tag="d",
                                                name="pf")
                h0_ = 0 if kind == "p1" else 4
                for hh in range(h0_, h0_ + 4):
                    nc.tensor.matmul(
                        ps[:],
                        ynorm[hh, ti // 8][:, ts(ti % 8, P)],
                        wpt[hh][:, ts(oc, CH)],
                        start=(hh == h0_),
                        stop=(hh == h0_ + 3),
                    )
                ob = pap_pool[0].tile([P, CH], BF16, tag="pa")
                nc.vector.tensor_copy(ob[:], ps[:])
                dst = outa if kind == "p1" else out
                nc.sync.dma_start(dst[ts(ti, P), ts(oc, CH)], ob[:])
                return True

            def pair_gen(pi):
                h, cp = pairs[pi]
                vc, vo = h // 4, (h % 4) * P
                y0 = psY.tile([P, CH], F32, tag="y", name="y0")
                y1 = psY.tile([P, CH], F32, tag="y", name="y1")
                s_tiles = s_store[pi]
                es = {}
                lvl = {}
                for kt in range(TT):
                    if kt == 3 and pend and psd_pool[0] is not None:
                        emit_tail(pend.popleft())
                    if kt == 9 and len(pend) > 1 and psd_pool[0] is not None:
                        emit_tail(pend.popleft())
                    if (kt % 2 == 1 and pi >= 4 and
                            len(fillq) > P1_RESERVE):
                        emit_fill()
                    epool = ep if pi < 2 else ep2_pool[0]
                    e = epool.tile([P, 2 * CH], BF16, tag="e")
                    nc.scalar.activation(
                        e[:], s_tiles.pop(kt)[:],
                        mybir.ActivationFunctionType.Exp, scale=SCALE,
                    )
                    es[kt] = e
                    if kt + 2 < TT:
                        s_tiles[kt + 2] = s_mm(pi, kt + 2)
                    elif pi + 1 < NP:
                        # emit the NEXT pair's first score groups early so
                        # the tail (av15 waiting on exp15) never blocks them
                        s_store[pi + 1][kt + 2 - TT] = s_mm(pi + 1, kt + 2 - TT)
                    elif len(fillq) > P1_RESERVE:
                        emit_fill()
                    vT = v_t[vc, kt][:, vo:vo + P]
                    nc.tensor.matmul(y0[:], vT, e[:, 0:CH],
                                     start=(kt == 0), stop=(kt == TT - 1))
                    nc.tensor.matmul(y1[:], vT, e[:, CH:2 * CH],
                                     start=(kt == 0), stop=(kt == TT - 1))
                    # denominator tree: bf16 pairwise adds on DVE
                    if kt % 2 == 1:
                        t1 = spool.tile([P, 2 * CH], BF16, tag="t")
                        nc.vector.tensor_add(t1[:], es.pop(kt - 1)[:],
                                             es.pop(kt)[:])
                        lvl[1, kt // 2] = t1
                    for L in (1, 2, 3):
                        j = (kt + 1) // (1 << (L + 1))
                        if (kt + 1) % (1 << (L + 1)) == 0:
                            t2 = spool.tile([P, 2 * CH], BF16, tag="t")
                            nc.vector.tensor_add(
                                t2[:], lvl.pop((L, 2 * j - 2))[:],
                                lvl.pop((L, 2 * j - 1))[:])
                            lvl[L + 1, j - 1] = t2
                    yield
                sfin = lvl.pop((4, 0))
                # free the y psum banks early so the next pair's AV
                # accumulation never waits on this pair's recip/mul;
                # post-k these copies ride the Scalar queue (DVE is the
                # attention phase's tightest engine)
                yc0 = ycp.tile([P, CH], BF16, tag="yc", name="yc0")
                yc1 = ycp.tile([P, CH], BF16, tag="yc", name="yc1")
                if pi < 2:
                    nc.vector.tensor_copy(yc0[:], y0[:])
                    nc.vector.tensor_copy(yc1[:], y1[:])
                else:
                    nc.scalar.copy(yc0[:], y0[:])
                    nc.scalar.copy(yc1[:], y1[:])
                pend.append((h, cp, sfin, yc0, yc1))

            def all_pairs():
                for pi in range(NP):
                    yield from pair_gen(pi)

            pump = all_pairs()
            pumped = [0]

            def pump_units(n):
                for _ in range(n):
                    if next(pump, StopIteration) is StopIteration:
                        return False
                    pumped[0] += 1
                return True

            # ---------------- Phase K (+ injected pairs 0-1) ----------------
            # From here on the Scalar queue belongs to the exp stream:
            # every DMA trigger costs ~0.65us of issuing-engine time and a
            # WAR-waiting trigger blocks the whole queue, so all further
            # loads/stores trigger from Sync only.
            started = [False]
            for ci in range(NCH):
                if ci == 1:
                    msk[2] = load_masks(2, sync_only=True)
                    xq[2] = load_x(2, sync_only=True)
                elif ci == 2:
                    msk[3] = load_masks(3, sync_only=True)
                    xq[3] = load_x(3, sync_only=True)
                cs, sn = msk[ci]
                for fi in range(HPC):
                    ps = ps_k.tile([P, CH], F32, tag="mmk")
                    for kt in range(KT):
                        nc.tensor.matmul(
                            ps[:],
                            wk_t[fi // 4][kt][:, ts(fi % 4, P)],
                            xq[ci][kt][:],
                            start=(kt == 0),
                            stop=(kt == KT - 1),
                        )
                    k_t[fi, ci] = rope(ps, f"k{fi}_{ci}", cs, sn)
                    # pump early-pair units once their k chunks exist:
                    # pair-0 unit kt needs k_t[0, (kt+2)//4] for its s_mm
                    if fi >= 1:
                        if not started[0] and ci == 0:
                            s_store[0][0] = s_mm(0, 0)
                            s_store[0][1] = s_mm(0, 1)
                            started[0] = True
                        # pair-0 unit kt emits s_mm(kt+2) needing k chunk
                        # (kt+2)//4; pair 1 (h1) consumes the same chunks.
                        # Cap at 24 so pair-1's tail lands in attention.
                        max_units = (4 * (ci + 1) - 2) if ci < 3 else 24
                        per_win = 2 if ci < 3 else 3
                        pump_units(min(per_win, max_units - pumped[0]))
            ps_k.release()
            es1.close()

            # ---------------- attention (remaining pairs) ----------------
            ynp = outer.enter_context(tc.tile_pool(name="ynorm", bufs=1))
            wpp = outer.enter_context(tc.tile_pool(name="wp", bufs=1))
            rcp = outer.enter_context(
                tc.tile_pool(name="rc", bufs=2, side="right"))
            pap = outer.enter_context(
                tc.tile_pool(name="pa", bufs=4, side="right"))
            ep2 = outer.enter_context(
                tc.tile_pool(name="ee2", bufs=8, side="right"))
            ep2_pool[0] = ep2
            psD = tc.alloc_tile_pool(name="psD", bufs=2, space="PSUM")
            psd_pool[0] = psD
            rcp_pool[0] = rcp
            pap_pool[0] = pap
            for h in range(HPC):
                for half in range(2):
                    ynorm[h, half] = ynp.tile(
                        [P, 2 * CH], BF16, tag=f"yn{h}_{half}",
                        name=f"ynorm{h}_{half}")
            for h in range(HPC):
                wtl = wpp.tile([P, C], BF16, tag=f"wp{h}", name=f"wpt{h}")
                nc.sync.dma_start(wtl[:], wpd[h])
                wpt.append(wtl)

            while pump_units(16):
                pass

            # ---------------- Phase 3: boundary + remaining out-proj ----
            # flush the last tails with the reserved fill groups covering
            # the recip/mul chain, then hand the freed PSUM banks to the
            # final p3 tranche (ti 8-15, which needs the last tail).
            emit_fill()
            emit_fill()
            while pend:
                emit_tail(pend.popleft())
            while fillq:
                emit_fill()
            psD.release()
            psY.release()
            psS.release()
            ps3 = tc.alloc_tile_pool(name="ps3", bufs=4, space="PSUM")
            while emit_fill(pool=ps3):
                pass
            ps3.release()

    nc.compile()
    return nc


def get_nc():
    global _CACHED_NC
    if _CACHED_NC is None:
        _CACHED_NC = build_nc()
    return _CACHED_NC


def make_rope_masks():
    half = D // 2
    inv = 1.0 / (ROPE_BASE ** (np.arange(half, dtype=np.float64) * 2.0 / D))
    ang = np.arange(T, dtype=np.float64)[:, None] * inv[None, :]  # [T, half]
    cos = np.cos(ang).T.astype(np.float32)  # [half, T]
    sin = np.sin(ang).T.astype(np.float32)
    cosm = np.empty((P, T), np.float32)
    sinm = np.empty((P, T), np.float32)
    cosm[0::2] = cos
    cosm[1::2] = cos
    sinm[0::2] = -sin
    sinm[1::2] = sin
    return cosm, sinm


def make_in_maps(x, w_attn, w_proj):
    import ml_dtypes
    BF = ml_dtypes.bfloat16

    x = np.asarray(x, dtype=np.float32)
    w_attn = np.asarray(w_attn, dtype=np.float32)
    w_proj = np.asarray(w_proj, dtype=np.float32)
    cosm, sinm = make_rope_masks()
    cosm16 = cosm.astype(BF)
    sinm16 = sinm.astype(BF)
    ones16 = np.ones((P, P), BF)
    in_maps = []
    for core in range(8):
        b, hg = core // 2, core % 2
        h0 = hg * HPC
        rq = slice(h0 * D, (h0 + HPC) * D)
        rk = slice(C + h0 * D, C + (h0 + HPC) * D)
        rv = slice(2 * C + h0 * D, 2 * C + (h0 + HPC) * D)
        # x tiles: [NCH, KT, P, CH] from x[b].T
        xt = np.ascontiguousarray(x[b].T.astype(BF))
        xtp = np.ascontiguousarray(
            xt.reshape(KT, P, NCH, CH).transpose(2, 0, 1, 3))
        # wq/wk/wv: [C, HPC*D] -> [KT, 2, P, CH] (1KB-row half tiles)
        def wtile(w):
            wT = w.T.astype(BF)  # [C, HPC*D]
            return np.ascontiguousarray(
                wT.reshape(KT, P, 2, CH).transpose(0, 2, 1, 3))
        wvd = wtile(w_attn[rv])
        wpT = np.ascontiguousarray(
            w_proj[:, h0 * D:(h0 + HPC) * D].T.astype(BF)).reshape(HPC, P, C)
        in_maps.append({
            "xtp": xtp,
            "wqd": wtile(w_attn[rq]),
            "wkd": wtile(w_attn[rk]),
            "wvd": wvd,
            "wpd": wpT,
            "cosm": cosm16,
            "sinm": sinm16,
            "onesd": ones16,
        })
    return in_maps


def combine_outputs(results):
    B = 4
    out = np.empty((B, T, C), np.float32)
    for b in range(B):
        out[b] = (results[2 * b]["out"].astype(np.float32)
                  + results[2 * b]["outa"].astype(np.float32)
                  + results[2 * b + 1]["out"].astype(np.float32)
                  + results[2 * b + 1]["outa"].astype(np.float32))
    return out


def kernel(x, w_attn, w_proj):
    from concourse.bass_utils import run_bass_kernel_spmd

    nc = get_nc()
    in_maps = make_in_maps(x, w_attn, w_proj)
    res = run_bass_kernel_spmd(nc, in_maps, list(range(8)))
    return combine_outputs(res.results)


# revision 54
# speedup vs baseline: 1.2490x; 1.0052x over previous
"""Trainium2 Bass kernel for nn_Attention_54778012893268.

Fused QKV projection + RoPE + non-causal SDPA + output projection.
B=4, T=2048, C=2048, H=16, D=128, fp32 in / bf16 partial out.

Sharding: 8 cores = (batch b, head-group hg) pairs; b = core//2, hg = core%2.
Each core handles one batch's tokens and 8 of the 16 heads end-to-end,
producing two partial [T, C] bf16 outputs (heads 0-3 and 4-7 of its
group); the host upcasts and sums the partials across cores.

Design (v8; v7 was 735us, ACT-bound in the attention phase):
- Projection order q -> v -> k, so by ~6% into the k phase head-0's
  q/k/v are all ready and the first two attention pairs (scores + exp +
  AV + denominator tree) are pumped INTO the k-phase window via a
  generator that yields one kt-unit at a time. The ACT exp stream
  (290us total) starts ~100us earlier than v7, so the attention phase
  is PE-bound instead of ACT-bound.
- x is NOT kept resident: a 32-slot ring re-loads it per phase
  (3x8MB of DMA, amortized), freeing 32KB/partition for the early-
  attention pools. All q/v/k weights share one 48-slot ring whose WAR
  dependencies sequence the prefetches naturally.
- every matmul operand is bf16 (PSUM f32); rel err ~9e-3 vs 2e-2 gate.
- softmax denominator: bf16 pairwise DVE tree over the 16 E tiles per
  pair + one ones-matmul per chunk; normalization deferred one pair
  (pend queue) so the PE FIFO never blocks on the DVE tail.
- ynorm is split per (head, T-half) so the final pair's normalization
  only gates the p3 tiles that actually read it; p3 ti 0-7 stream
  while pair-15's tail chain completes, and a reserve of p1 groups
  fills the remaining boundary bubble.
- DMA: all tensors host-pre-tiled to >=1KB rows; loads alternate
  between the two trigger queues (sync/scalar).
"""

import math
import sys
from collections import deque

import numpy as np

sys.path.insert(0, "/opt/trn_rl_repo")

P = 128
T = 2048
C = 2048
HPC = 8          # heads per core
D = 128
CH = 512         # T-chunk (PSUM bank width at fp32)
NCH = T // CH    # 4
KT = C // P      # 16 contraction tiles
TT = T // P      # 16 token tiles
SCALE = 1.0 / math.sqrt(D)
ROPE_BASE = 10000.0

WARMUP_MM = 380  # junk matmuls bridging the initial DMA ramp
P1_RESERVE = 2   # fill groups kept back to cover the attention->p3 boundary

_CACHED_NC = None


def build_nc():
    import concourse.bass as bass
    import concourse.tile as tile
    from concourse import bacc, mybir

    F32 = mybir.dt.float32
    BF16 = mybir.dt.bfloat16
    ts = bass.ts

    nc = bacc.Bacc("TRN2", target_bir_lowering=False, debug=False, num_devices=8)

    xtp = nc.dram_tensor("xtp", [NCH, KT, P, CH], BF16, kind="ExternalInput").ap()
    wqd = nc.dram_tensor("wqd", [KT, 2, P, CH], BF16, kind="ExternalInput").ap()
    wkd = nc.dram_tensor("wkd", [KT, 2, P, CH], BF16, kind="ExternalInput").ap()
    wvd = nc.dram_tensor("wvd", [KT, 2, P, CH], BF16, kind="ExternalInput").ap()
    wpd = nc.dram_tensor("wpd", [HPC, P, C], BF16, kind="ExternalInput").ap()
    cosm = nc.dram_tensor("cosm", [P, T], BF16, kind="ExternalInput").ap()
    sinm = nc.dram_tensor("sinm", [P, T], BF16, kind="ExternalInput").ap()
    onesd = nc.dram_tensor("onesd", [P, P], BF16, kind="ExternalInput").ap()
    out = nc.dram_tensor("out", [T, C], BF16, kind="ExternalOutput").ap()
    outa = nc.dram_tensor("outa", [T, C], BF16, kind="ExternalOutput").ap()

    # pair-swap shuffle mask (within each 32-partition quadrant)
    SWAP_MASK = [i ^ 1 for i in range(32)]

    with tile.TileContext(nc) as tc:
        from contextlib import ExitStack

        with ExitStack() as outer:
            cpool = outer.enter_context(tc.tile_pool(name="const", bufs=1))
            qkres = outer.enter_context(tc.tile_pool(name="qkres", bufs=1))
            vres = outer.enter_context(
                tc.tile_pool(name="vres", bufs=1, side="right"))

            ones = cpool.tile([P, P], BF16, tag="ones")
            nc.sync.dma_start(ones[:], onesd)

            # ---------------- projection scaffolding ----------------
            es1 = ExitStack()
            mpool = es1.enter_context(tc.tile_pool(name="masks", bufs=2))
            rp = es1.enter_context(tc.tile_pool(name="rope", bufs=2))
            wpool = es1.enter_context(tc.tile_pool(name="w", bufs=3 * KT))
            xpool = es1.enter_context(tc.tile_pool(name="xch", bufs=2 * KT))

            def load_masks(ci, sync_only=False):
                eng = nc.sync if sync_only else nc.scalar
                cs = mpool.tile([P, CH], BF16, tag="cos", name=f"cos{ci}")
                eng.dma_start(cs[:], cosm[:, ts(ci, CH)])
                sn = mpool.tile([P, CH], BF16, tag="sin", name=f"sin{ci}")
                nc.sync.dma_start(sn[:], sinm[:, ts(ci, CH)])
                return cs, sn

            # x ring: each load_x re-requests a chunk; the 32-slot ring's
            # WAR deps serialize against the previous consumer phase.
            dma_engs = None  # set after nc engines exist

            def load_x(ci, sync_only=False):
                tiles = []
                for kt in range(KT):
                    xtl = xpool.tile([P, CH], BF16, tag="x")
                    eng = dma_engs[0] if sync_only else dma_engs[kt % 2]
                    eng.dma_start(xtl[:], xtp[ci, kt])
                    tiles.append(xtl)
                return tiles

            # weight ring: q-h0, q-h1, v-h0 | v-h1, k-h0, k-h1 (wraps)
            def load_w(wt, w_dram, half):
                for kt in range(KT):
                    w0 = wpool.tile([P, CH], BF16, tag="w")
                    dma_engs[(kt + 1) % 2].dma_start(w0[:], w_dram[kt, half])
                    wt[half][kt] = w0

            wq_t = {0: {}, 1: {}}
            wv_t = {0: {}, 1: {}}
            wk_t = {0: {}, 1: {}}

            # ---- ramp: interleave weight + x loads across both queues ----
            dma_engs = (nc.sync, nc.scalar)
            load_w(wq_t, wqd, 0)
            msk = {0: load_masks(0)}
            xq = {0: load_x(0)}
            load_w(wq_t, wqd, 1)
            msk[1] = load_masks(1)
            xq[1] = load_x(1)
            load_w(wv_t, wvd, 0)   # fresh slots 32-47, no WAR

            ps_q = tc.alloc_tile_pool(name="psq", bufs=4, space="PSUM")

            # warm the PE HAM across the DMA ramp; the junk exp preloads
            # the ACT exp table before the attention phase
            warm_ps = ps_q.tile([P, 64], F32, tag="mm", name="warmps")
            for wi in range(WARMUP_MM):
                nc.tensor.matmul(warm_ps[:], ones[:], ones[:, :64],
                                 start=(wi == 0), stop=(wi == WARMUP_MM - 1))
            wexp = rp.tile([P, CH], BF16, tag="r0", name="warmexp")
            nc.scalar.activation(wexp[:, :64], warm_ps[:],
                                 mybir.ActivationFunctionType.Exp, scale=SCALE)

            q_t = {}   # (h, ci) -> [128 d, 512 t] bf16
            k_t = {}
            v_t = {}   # (vc, ti) -> [128 t, 512 f] bf16

            def rope(ps, tag, cs, sn):
                e0 = rp.tile([P, CH], BF16, tag="r0")
                nc.vector.tensor_copy(e0[:], ps[:])
                e1 = rp.tile([P, CH], BF16, tag="r1")
                nc.vector.stream_shuffle(e1[:], e0[:], SWAP_MASK)
                a = rp.tile([P, CH], BF16, tag="r1", name="ra")
                nc.vector.tensor_mul(a[:], e0[:], cs[:])
                b = rp.tile([P, CH], BF16, tag="r0", name="rb")
                nc.vector.tensor_mul(b[:], e1[:], sn[:])
                ro = qkres.tile([P, CH], BF16, tag=tag)
                nc.vector.tensor_add(ro[:], a[:], b[:])
                return ro

            # ---------------- Phase Q ----------------
            for ci in range(NCH):
                cs, sn = msk[ci]
                for fi in range(HPC):
                    ps = ps_q.tile([P, CH], F32, tag="mm")
                    for kt in range(KT):
                        nc.tensor.matmul(
                            ps[:],
                            wq_t[fi // 4][kt][:, ts(fi % 4, P)],
                            xq[ci][kt][:],
                            start=(kt == 0),
                            stop=(kt == KT - 1),
                        )
                    q_t[fi, ci] = rope(ps, f"q{fi}_{ci}", cs, sn)
                # schedule re-loads as ring slots free up
                if ci == 0:
                    msk[2] = load_masks(2)
                    xq[2] = load_x(2)
                elif ci == 1:
                    msk[3] = load_masks(3)
                    xq[3] = load_x(3)
                elif ci == 2:
                    xq[0] = load_x(0)       # for v phase
                    load_w(wv_t, wvd, 1)    # into q-h0 slots
                elif ci == 3:
                    xq[1] = load_x(1)
            ps_q.release()

            # ---------------- Phase V ----------------
            ps_v = tc.alloc_tile_pool(name="psv", bufs=4, space="PSUM")
            load_w(wk_t, wkd, 0)            # into q-h1 slots
            msk[0] = load_masks(0)          # masks for the k phase
            msk[1] = load_masks(1)
            for ci in range(NCH):
                for vc in range(2):
                    for sub in range(4):
                        ti = 4 * ci + sub
                        ps = ps_v.tile([P, CH], F32, tag="mmv")
                        for kt in range(KT):
                            nc.tensor.matmul(
                                ps[:],
                                xq[ci][kt][:, ts(sub, P)],
                                wv_t[vc][kt][:],
                                start=(kt == 0),
                                stop=(kt == KT - 1),
                            )
                        sb = vres.tile([P, CH], BF16, tag=f"v{vc}_{ti}")
                        nc.vector.tensor_copy(sb[:], ps[:])
                        v_t[vc, ti] = sb
                if ci == 0:
                    xq[2] = load_x(2)
                elif ci == 1:
                    xq[3] = load_x(3)
                elif ci == 2:
                    xq[0] = load_x(0)       # for k phase
                elif ci == 3:
                    xq[1] = load_x(1)
            ps_v.release()
            load_w(wk_t, wkd, 1)            # into v-h0 slots (free at v end)

            # ------------- attention pools (open before k) -------------
            ep = outer.enter_context(
                tc.tile_pool(name="ee", bufs=4, side="right"))
            spool = outer.enter_context(
                tc.tile_pool(name="st", bufs=5, side="right"))
            ycp = outer.enter_context(
                tc.tile_pool(name="yc", bufs=4, side="right"))

            psS = tc.alloc_tile_pool(name="psS", bufs=2, space="PSUM")
            psY = tc.alloc_tile_pool(name="psY", bufs=2, space="PSUM")
            ps_k = tc.alloc_tile_pool(name="psk", bufs=2, space="PSUM")

            # ---------------- attention machinery ----------------
            # cp-major order within each head quad: out-projection tranches
            # unlock progressively (p1 ti0-7 after pair 3, p1 ti8-15 after
            # pair 7, p3 ti0-7 after pair 11), giving the ACT-deficit pairs
            # PE filler work throughout.
            pairs = ([(h, 0) for h in range(4)] + [(h, 1) for h in range(4)]
                     + [(h, 0) for h in range(4, 8)]
                     + [(h, 1) for h in range(4, 8)])
            NP = len(pairs)
            s_store = {pi: {} for pi in range(NP)}
            pend = deque()
            psd_pool = [None]   # psD opens after ps_k closes
            ep2_pool = [None]   # wider E ring, opens after k frees SBUF
            ynorm = {}          # (h, half) -> [128 d, 1024 t] bf16
            wpt = []
            rcp_pool = [None]
            pap_pool = [None]

            def s_mm(pi, kt):
                h, cp = pairs[pi]
                sp = psS.tile([P, 2 * CH], F32, tag="s", name=f"s{kt}")
                kT = k_t[h, kt // 4][:, ts(kt % 4, P)]
                nc.tensor.matmul(sp[:, 0:CH], kT, q_t[h, 2 * cp][:],
                                 start=True, stop=True)
                nc.tensor.matmul(sp[:, CH:2 * CH], kT, q_t[h, 2 * cp + 1][:],
                                 start=True, stop=True)
                return sp

            tails_done = [0]
            boundary = [0]

            def emit_tail(t):
                h_, cp_, sfin, yc0, yc1 = t
                psD = psd_pool[0]
                rcp = rcp_pool[0]
                yn = ynorm[h_, cp_]
                d0 = psD.tile([P, CH], F32, tag="d", name="d0")
                nc.tensor.matmul(d0[:], ones[:], sfin[:, 0:CH],
                                 start=True, stop=True)
                d1 = psD.tile([P, CH], F32, tag="d", name="d1")
                nc.tensor.matmul(d1[:], ones[:], sfin[:, CH:2 * CH],
                                 start=True, stop=True)
                r0 = rcp.tile([P, CH], F32, tag="rc")
                nc.vector.reciprocal_approx_fast(r0[:], d0[:])
                nc.vector.tensor_mul(yn[:, 0:CH], yc0[:], r0[:])
                r1 = rcp.tile([P, CH], F32, tag="rc")
                nc.vector.reciprocal_approx_fast(r1[:], d1[:])
                nc.vector.tensor_mul(yn[:, CH:2 * CH], yc1[:], r1[:])
                tails_done[0] += 1

            # out-projection filler: partial A (heads 0-3 -> outa) and
            # partial B (heads 4-7 -> out), streamed into the attention
            # phase's ACT-deficit PE slots as their ynorm tranches unlock.
            fillq = ([("p1", ti, oc) for ti in range(TT) for oc in range(NCH)]
                     + [("p3", ti, oc) for ti in range(8) for oc in range(NCH)])
            p3q = [("p3", ti, oc) for ti in range(8, TT) for oc in range(NCH)]

            def fill_ready(ent):
                kind, ti, oc = ent
                need = (4 if ti < 8 else 8) if kind == "p1" else \
                    (12 if ti < 8 else 16)
                return tails_done[0] >= need

            def emit_fill(pool=None):
                if not fillq:
                    if not p3q:
                        return False
                    ent = p3q.pop(0)
                elif fill_ready(fillq[0]):
                    ent = fillq.pop(0)
                else:
                    return False
                kind, ti, oc = ent
                ps = (pool or psd_pool[0]).tile([P, CH], F32, tag="d",
                                                name="pf")
                h0_ = 0 if kind == "p1" else 4
                for hh in range(h0_, h0_ + 4):
                    nc.tensor.matmul(
                        ps[:],
                        ynorm[hh, ti // 8][:, ts(ti % 8, P)],
                        wpt[hh][:, ts(oc, CH)],
                        start=(hh == h0_),
                        stop=(hh == h0_ + 3),
                    )
                ob = pap_pool[0].tile([P, CH], BF16, tag="pa")
                nc.vector.tensor_copy(ob[:], ps[:])
                dst = outa if kind == "p1" else out
                # during attention only Sync may trigger (Scalar = exp
                # stream); once exps are done, alternate both queues
                if boundary[0]:
                    boundary[0] += 1
                    eng = nc.sync if boundary[0] % 2 == 0 else nc.scalar
                else:
                    eng = nc.sync
                eng.dma_start(dst[ts(ti, P), ts(oc, CH)], ob[:])
                return True

            def pair_gen(pi):
                h, cp = pairs[pi]
                vc, vo = h // 4, (h % 4) * P
                y0 = psY.tile([P, CH], F32, tag="y", name="y0")
                y1 = psY.tile([P, CH], F32, tag="y", name="y1")
                s_tiles = s_store[pi]
                es = {}
                lvl = {}
                for kt in range(TT):
                    if kt == 3 and pend and psd_pool[0] is not None:
                        emit_tail(pend.popleft())
                    if kt == 9 and len(pend) > 1 and psd_pool[0] is not None:
                        emit_tail(pend.popleft())
                    if (kt % 2 == 1 and pi >= 4 and
                            len(fillq) > P1_RESERVE):
                        emit_fill()
                    epool = ep if pi < 2 else ep2_pool[0]
                    e = epool.tile([P, 2 * CH], BF16, tag="e")
                    nc.scalar.activation(
                        e[:], s_tiles.pop(kt)[:],
                        mybir.ActivationFunctionType.Exp, scale=SCALE,
                    )
                    es[kt] = e
                    if kt + 2 < TT:
                        s_tiles[kt + 2] = s_mm(pi, kt + 2)
                    elif pi + 1 < NP:
                        # emit the NEXT pair's first score groups early so
                        # the tail (av15 waiting on exp15) never blocks them
                        s_store[pi + 1][kt + 2 - TT] = s_mm(pi + 1, kt + 2 - TT)
                    elif len(fillq) > P1_RESERVE:
                        emit_fill()
                    vT = v_t[vc, kt][:, vo:vo + P]
                    nc.tensor.matmul(y0[:], vT, e[:, 0:CH],
                                     start=(kt == 0), stop=(kt == TT - 1))
                    nc.tensor.matmul(y1[:], vT, e[:, CH:2 * CH],
                                     start=(kt == 0), stop=(kt == TT - 1))
                    # denominator tree: bf16 pairwise adds on DVE
                    if kt % 2 == 1:
                        t1 = spool.tile([P, 2 * CH], BF16, tag="t")
                        nc.vector.tensor_add(t1[:], es.pop(kt - 1)[:],
                                             es.pop(kt)[:])
                        lvl[1, kt // 2] = t1
                    for L in (1, 2, 3):
                        j = (kt + 1) // (1 << (L + 1))
                        if (kt + 1) % (1 << (L + 1)) == 0:
                            t2 = spool.tile([P, 2 * CH], BF16, tag="t")
                            nc.vector.tensor_add(
                                t2[:], lvl.pop((L, 2 * j - 2))[:],
                                lvl.pop((L, 2 * j - 1))[:])
                            lvl[L + 1, j - 1] = t2
                    yield
                sfin = lvl.pop((4, 0))
                # free the y psum banks early so the next pair's AV
                # accumulation never waits on this pair's recip/mul;
                # post-k these copies ride the Scalar queue (DVE is the
                # attention phase's tightest engine)
                yc0 = ycp.tile([P, CH], BF16, tag="yc", name="yc0")
                yc1 = ycp.tile([P, CH], BF16, tag="yc", name="yc1")
                if pi < 2:
                    nc.vector.tensor_copy(yc0[:], y0[:])
                    nc.vector.tensor_copy(yc1[:], y1[:])
                else:
                    nc.scalar.copy(yc0[:], y0[:])
                    nc.scalar.copy(yc1[:], y1[:])
                pend.append((h, cp, sfin, yc0, yc1))

            def all_pairs():
                for pi in range(NP):
                    yield from pair_gen(pi)

            pump = all_pairs()
            pumped = [0]

            def pump_units(n):
                for _ in range(n):
                    if next(pump, StopIteration) is StopIteration:
                        return False
                    pumped[0] += 1
                return True

            # ---------------- Phase K (+ injected pairs 0-1) ----------------
            # From here on the Scalar queue belongs to the exp stream:
            # every DMA trigger costs ~0.65us of issuing-engine time and a
            # WAR-waiting trigger blocks the whole queue, so all further
            # loads/stores trigger from Sync only.
            started = [False]
            for ci in range(NCH):
                if ci == 1:
                    msk[2] = load_masks(2, sync_only=True)
                    xq[2] = load_x(2, sync_only=True)
                elif ci == 2:
                    msk[3] = load_masks(3, sync_only=True)
                    xq[3] = load_x(3, sync_only=True)
                cs, sn = msk[ci]
                for fi in range(HPC):
                    ps = ps_k.tile([P, CH], F32, tag="mmk")
                    for kt in range(KT):
                        nc.tensor.matmul(
                            ps[:],
                            wk_t[fi // 4][kt][:, ts(fi % 4, P)],
                            xq[ci][kt][:],
                            start=(kt == 0),
                            stop=(kt == KT - 1),
                        )
                    k_t[fi, ci] = rope(ps, f"k{fi}_{ci}", cs, sn)
                    # pump early-pair units once their k chunks exist:
                    # pair-0 unit kt needs k_t[0, (kt+2)//4] for its s_mm
                    if fi >= 1:
                        if not started[0] and ci == 0:
                            s_store[0][0] = s_mm(0, 0)
                            s_store[0][1] = s_mm(0, 1)
                            started[0] = True
                        # pair-0 unit kt emits s_mm(kt+2) needing k chunk
                        # (kt+2)//4; pair 1 (h1) consumes the same chunks.
                        # Cap at 24 so pair-1's tail lands in attention.
                        max_units = (4 * (ci + 1) - 2) if ci < 3 else 24
                        per_win = 2 if ci < 3 else 3
                        pump_units(min(per_win, max_units - pumped[0]))
            ps_k.release()
            es1.close()

            # ---------------- attention (remaining pairs) ----------------
            ynp = outer.enter_context(tc.tile_pool(name="ynorm", bufs=1))
            wpp = outer.enter_context(tc.tile_pool(name="wp", bufs=1))
            rcp = outer.enter_context(
                tc.tile_pool(name="rc", bufs=2, side="right"))
            pap = outer.enter_context(
                tc.tile_pool(name="pa", bufs=6, side="right"))
            ep2 = outer.enter_context(
                tc.tile_pool(name="ee2", bufs=7, side="right"))
            ep2_pool[0] = ep2
            psD = tc.alloc_tile_pool(name="psD", bufs=2, space="PSUM")
            psd_pool[0] = psD
            rcp_pool[0] = rcp
            pap_pool[0] = pap
            for h in range(HPC):
                for half in range(2):
                    ynorm[h, half] = ynp.tile(
                        [P, 2 * CH], BF16, tag=f"yn{h}_{half}",
                        name=f"ynorm{h}_{half}")
            for h in range(HPC):
                wtl = wpp.tile([P, C], BF16, tag=f"wp{h}", name=f"wpt{h}")
                nc.sync.dma_start(wtl[:], wpd[h])
                wpt.append(wtl)

            while pump_units(16):
                pass

            # ---------------- Phase 3: boundary + remaining out-proj ----
            # flush the last tails with the reserved fill groups covering
            # the recip/mul chain, then hand the freed PSUM banks to the
            # final p3 tranche (ti 8-15, which needs the last tail).
            boundary[0] = 1
            emit_fill()
            emit_fill()
            while pend:
                emit_tail(pend.popleft())
            while fillq:
                emit_fill()
            psD.release()
            psY.release()
            psS.release()
            ps3 = tc.alloc_tile_pool(name="ps3", bufs=4, space="PSUM")
            while emit_fill(pool=ps3):
                pass
            ps3.release()

    nc.compile()
    return nc


def get_nc():
    global _CACHED_NC
    if _CACHED_NC is None:
        _CACHED_NC = build_nc()
    return _CACHED_NC


def make_rope_masks():
    half = D // 2
    inv = 1.0 / (ROPE_BASE ** (np.arange(half, dtype=np.float64) * 2.0 / D))
    ang = np.arange(T, dtype=np.float64)[:, None] * inv[None, :]  # [T, half]
    cos = np.cos(ang).T.astype(np.float32)  # [half, T]
    sin = np.sin(ang).T.astype(np.float32)
    cosm = np.empty((P, T), np.float32)
    sinm = np.empty((P, T), np.float32)
    cosm[0::2] = cos
    cosm[1::2] = cos
    sinm[0::2] = -sin
    sinm[1::2] = sin
    return cosm, sinm


def make_in_maps(x, w_attn, w_proj):
    import ml_dtypes
    BF = ml_dtypes.bfloat16

    x = np.asarray(x, dtype=np.float32)
    w_attn = np.asarray(w_attn, dtype=np.float32)
    w_proj = np.asarray(w_proj, dtype=np.float32)
    cosm, sinm = make_rope_masks()
    cosm16 = cosm.astype(BF)
    sinm16 = sinm.astype(BF)
    ones16 = np.ones((P, P), BF)
    in_maps = []
    for core in range(8):
        b, hg = core // 2, core % 2
        h0 = hg * HPC
        rq = slice(h0 * D, (h0 + HPC) * D)
        rk = slice(C + h0 * D, C + (h0 + HPC) * D)
        rv = slice(2 * C + h0 * D, 2 * C + (h0 + HPC) * D)
        # x tiles: [NCH, KT, P, CH] from x[b].T
        xt = np.ascontiguousarray(x[b].T.astype(BF))
        xtp = np.ascontiguousarray(
            xt.reshape(KT, P, NCH, CH).transpose(2, 0, 1, 3))
        # wq/wk/wv: [C, HPC*D] -> [KT, 2, P, CH] (1KB-row half tiles)
        def wtile(w):
            wT = w.T.astype(BF)  # [C, HPC*D]
            return np.ascontiguousarray(
                wT.reshape(KT, P, 2, CH).transpose(0, 2, 1, 3))
        wvd = wtile(w_attn[rv])
        wpT = np.ascontiguousarray(
            w_proj[:, h0 * D:(h0 + HPC) * D].T.astype(BF)).reshape(HPC, P, C)
        in_maps.append({
            "xtp": xtp,
            "wqd": wtile(w_attn[rq]),
            "wkd": wtile(w_attn[rk]),
            "wvd": wvd,
            "wpd": wpT,
            "cosm": cosm16,
            "sinm": sinm16,
            "onesd": ones16,
        })
    return in_maps


def combine_outputs(results):
    B = 4
    out = np.empty((B, T, C), np.float32)
    for b in range(B):
        out[b] = (results[2 * b]["out"].astype(np.float32)
                  + results[2 * b]["outa"].astype(np.float32)
                  + results[2 * b + 1]["out"].astype(np.float32)
                  + results[2 * b + 1]["outa"].astype(np.float32))
    return out


def kernel(x, w_attn, w_proj):
    from concourse.bass_utils import run_bass_kernel_spmd

    nc = get_nc()
    in_maps = make_in_maps(x, w_attn, w_proj)
    res = run_bass_kernel_spmd(nc, in_maps, list(range(8)))
    return combine_outputs(res.results)


# revision 57
# speedup vs baseline: 1.2498x; 1.0006x over previous
"""Trainium2 Bass kernel for nn_Attention_54778012893268.

Fused QKV projection + RoPE + non-causal SDPA + output projection.
B=4, T=2048, C=2048, H=16, D=128, fp32 in / bf16 partial out.

Sharding: 8 cores = (batch b, head-group hg) pairs; b = core//2, hg = core%2.
Each core handles one batch's tokens and 8 of the 16 heads end-to-end,
producing two partial [T, C] bf16 outputs (heads 0-3 and 4-7 of its
group); the host upcasts and sums the partials across cores.

Design (v8; v7 was 735us, ACT-bound in the attention phase):
- Projection order q -> v -> k, so by ~6% into the k phase head-0's
  q/k/v are all ready and the first two attention pairs (scores + exp +
  AV + denominator tree) are pumped INTO the k-phase window via a
  generator that yields one kt-unit at a time. The ACT exp stream
  (290us total) starts ~100us earlier than v7, so the attention phase
  is PE-bound instead of ACT-bound.
- x is NOT kept resident: a 32-slot ring re-loads it per phase
  (3x8MB of DMA, amortized), freeing 32KB/partition for the early-
  attention pools. All q/v/k weights share one 48-slot ring whose WAR
  dependencies sequence the prefetches naturally.
- every matmul operand is bf16 (PSUM f32); rel err ~9e-3 vs 2e-2 gate.
- softmax denominator: bf16 pairwise DVE tree over the 16 E tiles per
  pair + one ones-matmul per chunk; normalization deferred one pair
  (pend queue) so the PE FIFO never blocks on the DVE tail.
- ynorm is split per (head, T-half) so the final pair's normalization
  only gates the p3 tiles that actually read it; p3 ti 0-7 stream
  while pair-15's tail chain completes, and a reserve of p1 groups
  fills the remaining boundary bubble.
- DMA: all tensors host-pre-tiled to >=1KB rows; loads alternate
  between the two trigger queues (sync/scalar).
"""

import math
import sys
from collections import deque

import numpy as np

sys.path.insert(0, "/opt/trn_rl_repo")

P = 128
T = 2048
C = 2048
HPC = 8          # heads per core
D = 128
CH = 512         # T-chunk (PSUM bank width at fp32)
NCH = T // CH    # 4
KT = C // P      # 16 contraction tiles
TT = T // P      # 16 token tiles
SCALE = 1.0 / math.sqrt(D)
ROPE_BASE = 10000.0

WARMUP_MM = 420  # junk matmuls bridging the initial DMA ramp
P1_RESERVE = 5   # fill groups kept back to cover the attention->p3 boundary

_CACHED_NC = None


def build_nc():
    import concourse.bass as bass
    import concourse.tile as tile
    from concourse import bacc, mybir

    F32 = mybir.dt.float32
    BF16 = mybir.dt.bfloat16
    ts = bass.ts

    nc = bacc.Bacc("TRN2", target_bir_lowering=False, debug=False, num_devices=8)

    xtp = nc.dram_tensor("xtp", [NCH, KT, P, CH], BF16, kind="ExternalInput").ap()
    wqd = nc.dram_tensor("wqd", [KT, 2, P, CH], BF16, kind="ExternalInput").ap()
    wkd = nc.dram_tensor("wkd", [KT, 2, P, CH], BF16, kind="ExternalInput").ap()
    wvd = nc.dram_tensor("wvd", [KT, 2, P, CH], BF16, kind="ExternalInput").ap()
    wpd = nc.dram_tensor("wpd", [HPC, P, C], BF16, kind="ExternalInput").ap()
    cosm = nc.dram_tensor("cosm", [P, T], BF16, kind="ExternalInput").ap()
    sinm = nc.dram_tensor("sinm", [P, T], BF16, kind="ExternalInput").ap()
    onesd = nc.dram_tensor("onesd", [P, P], BF16, kind="ExternalInput").ap()
    out = nc.dram_tensor("out", [T, C], BF16, kind="ExternalOutput").ap()
    outa = nc.dram_tensor("outa", [T, C], BF16, kind="ExternalOutput").ap()

    # pair-swap shuffle mask (within each 32-partition quadrant)
    SWAP_MASK = [i ^ 1 for i in range(32)]

    with tile.TileContext(nc) as tc:
        from contextlib import ExitStack

        with ExitStack() as outer:
            cpool = outer.enter_context(tc.tile_pool(name="const", bufs=1))
            qkres = outer.enter_context(tc.tile_pool(name="qkres", bufs=1))
            vres = outer.enter_context(
                tc.tile_pool(name="vres", bufs=1, side="right"))

            ones = cpool.tile([P, P], BF16, tag="ones")
            nc.sync.dma_start(ones[:], onesd)

            # ---------------- projection scaffolding ----------------
            es1 = ExitStack()
            mpool = es1.enter_context(tc.tile_pool(name="masks", bufs=2))
            rp = es1.enter_context(tc.tile_pool(name="rope", bufs=2))
            wpool = es1.enter_context(tc.tile_pool(name="w", bufs=3 * KT))
            xpool = es1.enter_context(tc.tile_pool(name="xch", bufs=2 * KT))

            def load_masks(ci, sync_only=False):
                eng = nc.sync if sync_only else nc.scalar
                cs = mpool.tile([P, CH], BF16, tag="cos", name=f"cos{ci}")
                eng.dma_start(cs[:], cosm[:, ts(ci, CH)])
                sn = mpool.tile([P, CH], BF16, tag="sin", name=f"sin{ci}")
                nc.sync.dma_start(sn[:], sinm[:, ts(ci, CH)])
                return cs, sn

            # x ring: each load_x re-requests a chunk; the 32-slot ring's
            # WAR deps serialize against the previous consumer phase.
            dma_engs = None  # set after nc engines exist

            def load_x(ci, sync_only=False):
                tiles = []
                for kt in range(KT):
                    xtl = xpool.tile([P, CH], BF16, tag="x")
                    eng = dma_engs[0] if sync_only else dma_engs[kt % 2]
                    eng.dma_start(xtl[:], xtp[ci, kt])
                    tiles.append(xtl)
                return tiles

            # weight ring: q-h0, q-h1, v-h0 | v-h1, k-h0, k-h1 (wraps)
            def load_w(wt, w_dram, half):
                for kt in range(KT):
                    w0 = wpool.tile([P, CH], BF16, tag="w")
                    dma_engs[(kt + 1) % 2].dma_start(w0[:], w_dram[kt, half])
                    wt[half][kt] = w0

            wq_t = {0: {}, 1: {}}
            wv_t = {0: {}, 1: {}}
            wk_t = {0: {}, 1: {}}

            # ---- ramp: interleave weight + x loads across both queues ----
            dma_engs = (nc.sync, nc.scalar)
            load_w(wq_t, wqd, 0)
            msk = {0: load_masks(0)}
            xq = {0: load_x(0)}
            load_w(wq_t, wqd, 1)
            msk[1] = load_masks(1)
            xq[1] = load_x(1)
            load_w(wv_t, wvd, 0)   # fresh slots 32-47, no WAR

            ps_q = tc.alloc_tile_pool(name="psq", bufs=4, space="PSUM")

            # warm the PE HAM across the DMA ramp; the junk exp preloads
            # the ACT exp table before the attention phase
            warm_ps = ps_q.tile([P, 64], F32, tag="mm", name="warmps")
            for wi in range(WARMUP_MM):
                nc.tensor.matmul(warm_ps[:], ones[:], ones[:, :64],
                                 start=(wi == 0), stop=(wi == WARMUP_MM - 1))
            wexp = rp.tile([P, CH], BF16, tag="r0", name="warmexp")
            nc.scalar.activation(wexp[:, :64], warm_ps[:],
                                 mybir.ActivationFunctionType.Exp, scale=SCALE)

            q_t = {}   # (h, ci) -> [128 d, 512 t] bf16
            k_t = {}
            v_t = {}   # (vc, ti) -> [128 t, 512 f] bf16

            def rope(ps, tag, cs, sn):
                e0 = rp.tile([P, CH], BF16, tag="r0")
                nc.vector.tensor_copy(e0[:], ps[:])
                e1 = rp.tile([P, CH], BF16, tag="r1")
                nc.vector.stream_shuffle(e1[:], e0[:], SWAP_MASK)
                a = rp.tile([P, CH], BF16, tag="r1", name="ra")
                nc.vector.tensor_mul(a[:], e0[:], cs[:])
                b = rp.tile([P, CH], BF16, tag="r0", name="rb")
                nc.vector.tensor_mul(b[:], e1[:], sn[:])
                ro = qkres.tile([P, CH], BF16, tag=tag)
                nc.vector.tensor_add(ro[:], a[:], b[:])
                return ro

            # ---------------- Phase Q ----------------
            for ci in range(NCH):
                cs, sn = msk[ci]
                for fi in range(HPC):
                    ps = ps_q.tile([P, CH], F32, tag="mm")
                    for kt in range(KT):
                        nc.tensor.matmul(
                            ps[:],
                            wq_t[fi // 4][kt][:, ts(fi % 4, P)],
                            xq[ci][kt][:],
                            start=(kt == 0),
                            stop=(kt == KT - 1),
                        )
                    q_t[fi, ci] = rope(ps, f"q{fi}_{ci}", cs, sn)
                    # wv half-1 reuses the wq-h0 slots, whose last reader
                    # is chunk-3's fi==3 group: issue right after it so
                    # the trigger's WAR wait is short
                    if ci == 3 and fi == 3:
                        load_w(wv_t, wvd, 1)
                # schedule re-loads as ring slots free up
                if ci == 0:
                    msk[2] = load_masks(2)
                    xq[2] = load_x(2)
                elif ci == 1:
                    msk[3] = load_masks(3)
                    xq[3] = load_x(3)
                elif ci == 2:
                    xq[0] = load_x(0)       # for v phase
                elif ci == 3:
                    xq[1] = load_x(1)
            ps_q.release()

            # ---------------- Phase V ----------------
            ps_v = tc.alloc_tile_pool(name="psv", bufs=4, space="PSUM")
            load_w(wk_t, wkd, 0)            # into q-h1 slots
            msk[0] = load_masks(0)          # masks for the k phase
            msk[1] = load_masks(1)
            for ci in range(NCH):
                for vc in range(2):
                    for sub in range(4):
                        ti = 4 * ci + sub
                        ps = ps_v.tile([P, CH], F32, tag="mmv")
                        for kt in range(KT):
                            nc.tensor.matmul(
                                ps[:],
                                xq[ci][kt][:, ts(sub, P)],
                                wv_t[vc][kt][:],
                                start=(kt == 0),
                                stop=(kt == KT - 1),
                            )
                        sb = vres.tile([P, CH], BF16, tag=f"v{vc}_{ti}")
                        nc.vector.tensor_copy(sb[:], ps[:])
                        v_t[vc, ti] = sb
                if ci == 0:
                    xq[2] = load_x(2)
                elif ci == 1:
                    xq[3] = load_x(3)
                elif ci == 2:
                    xq[0] = load_x(0)       # for k phase
                elif ci == 3:
                    xq[1] = load_x(1)
            ps_v.release()
            load_w(wk_t, wkd, 1)            # into v-h0 slots (free at v end)

            # ------------- attention pools (open before k) -------------
            ep = outer.enter_context(
                tc.tile_pool(name="ee", bufs=4, side="right"))
            spool = outer.enter_context(
                tc.tile_pool(name="st", bufs=5, side="right"))
            ycp = outer.enter_context(
                tc.tile_pool(name="yc", bufs=4, side="right"))

            psS = tc.alloc_tile_pool(name="psS", bufs=2, space="PSUM")
            psY = tc.alloc_tile_pool(name="psY", bufs=2, space="PSUM")
            ps_k = tc.alloc_tile_pool(name="psk", bufs=2, space="PSUM")

            # ---------------- attention machinery ----------------
            # cp-major order within each head quad: out-projection tranches
            # unlock progressively (p1 ti0-7 after pair 3, p1 ti8-15 after
            # pair 7, p3 ti0-7 after pair 11), giving the ACT-deficit pairs
            # PE filler work throughout.
            pairs = ([(h, 0) for h in range(4)] + [(h, 1) for h in range(4)]
                     + [(h, 0) for h in range(4, 8)]
                     + [(h, 1) for h in range(4, 8)])
            NP = len(pairs)
            s_store = {pi: {} for pi in range(NP)}
            pend = deque()
            psd_pool = [None]   # psD opens after ps_k closes
            ep2_pool = [None]   # wider E ring, opens after k frees SBUF
            ynorm = {}          # (h, half) -> [128 d, 1024 t] bf16
            wpt = []
            rcp_pool = [None]
            pap_pool = [None]

            def s_mm(pi, kt):
                h, cp = pairs[pi]
                sp = psS.tile([P, 2 * CH], F32, tag="s", name=f"s{kt}")
                kT = k_t[h, kt // 4][:, ts(kt % 4, P)]
                nc.tensor.matmul(sp[:, 0:CH], kT, q_t[h, 2 * cp][:],
                                 start=True, stop=True)
                nc.tensor.matmul(sp[:, CH:2 * CH], kT, q_t[h, 2 * cp + 1][:],
                                 start=True, stop=True)
                return sp

            tails_done = [0]
            boundary = [0]

            def emit_tail(t):
                h_, cp_, sfin, yc0, yc1 = t
                psD = psd_pool[0]
                rcp = rcp_pool[0]
                yn = ynorm[h_, cp_]
                d0 = psD.tile([P, CH], F32, tag="d", name="d0")
                nc.tensor.matmul(d0[:], ones[:], sfin[:, 0:CH],
                                 start=True, stop=True)
                d1 = psD.tile([P, CH], F32, tag="d", name="d1")
                nc.tensor.matmul(d1[:], ones[:], sfin[:, CH:2 * CH],
                                 start=True, stop=True)
                r0 = rcp.tile([P, CH], F32, tag="rc")
                nc.vector.reciprocal_approx_fast(r0[:], d0[:])
                nc.vector.tensor_mul(yn[:, 0:CH], yc0[:], r0[:])
                r1 = rcp.tile([P, CH], F32, tag="rc")
                nc.vector.reciprocal_approx_fast(r1[:], d1[:])
                nc.vector.tensor_mul(yn[:, CH:2 * CH], yc1[:], r1[:])
                tails_done[0] += 1

            # out-projection filler: partial A (heads 0-3 -> outa) and
            # partial B (heads 4-7 -> out), streamed into the attention
            # phase's ACT-deficit PE slots as their ynorm tranches unlock.
            fillq = ([("p1", ti, oc) for ti in range(TT) for oc in range(NCH)]
                     + [("p3", ti, oc) for ti in range(8) for oc in range(NCH)])
            p3q = [("p3", ti, oc) for ti in range(8, TT) for oc in range(NCH)]

            def fill_ready(ent):
                kind, ti, oc = ent
                need = (4 if ti < 8 else 8) if kind == "p1" else \
                    (12 if ti < 8 else 16)
                return tails_done[0] >= need

            def emit_fill(pool=None):
                if not fillq:
                    if not p3q:
                        return False
                    ent = p3q.pop(0)
                elif fill_ready(fillq[0]):
                    ent = fillq.pop(0)
                else:
                    return False
                kind, ti, oc = ent
                ps = (pool or psd_pool[0]).tile([P, CH], F32, tag="d",
                                                name="pf")
                h0_ = 0 if kind == "p1" else 4
                for hh in range(h0_, h0_ + 4):
                    nc.tensor.matmul(
                        ps[:],
                        ynorm[hh, ti // 8][:, ts(ti % 8, P)],
                        wpt[hh][:, ts(oc, CH)],
                        start=(hh == h0_),
                        stop=(hh == h0_ + 3),
                    )
                ob = pap_pool[0].tile([P, CH], BF16, tag="pa")
                nc.vector.tensor_copy(ob[:], ps[:])
                dst = outa if kind == "p1" else out
                # during attention only Sync may trigger (Scalar = exp
                # stream); once exps are done, alternate both queues
                if boundary[0]:
                    boundary[0] += 1
                    eng = nc.sync if boundary[0] % 2 == 0 else nc.scalar
                else:
                    eng = nc.sync
                eng.dma_start(dst[ts(ti, P), ts(oc, CH)], ob[:])
                return True

            def pair_gen(pi):
                h, cp = pairs[pi]
                vc, vo = h // 4, (h % 4) * P
                y0 = psY.tile([P, CH], F32, tag="y", name="y0")
                y1 = psY.tile([P, CH], F32, tag="y", name="y1")
                s_tiles = s_store[pi]
                es = {}
                lvl = {}
                for kt in range(TT):
                    if kt == 3 and pend and psd_pool[0] is not None:
                        emit_tail(pend.popleft())
                    if kt == 9 and len(pend) > 1 and psd_pool[0] is not None:
                        emit_tail(pend.popleft())
                    if (kt % 2 == 1 and pi >= 4 and
                            len(fillq) > P1_RESERVE):
                        emit_fill()
                    epool = ep if pi < 2 else ep2_pool[0]
                    e = epool.tile([P, 2 * CH], BF16, tag="e")
                    nc.scalar.activation(
                        e[:], s_tiles.pop(kt)[:],
                        mybir.ActivationFunctionType.Exp, scale=SCALE,
                    )
                    es[kt] = e
                    if kt + 2 < TT:
                        s_tiles[kt + 2] = s_mm(pi, kt + 2)
                    elif pi + 1 < NP:
                        # emit the NEXT pair's first score groups early so
                        # the tail (av15 waiting on exp15) never blocks them
                        s_store[pi + 1][kt + 2 - TT] = s_mm(pi + 1, kt + 2 - TT)
                    elif len(fillq) > P1_RESERVE:
                        emit_fill()
                    vT = v_t[vc, kt][:, vo:vo + P]
                    nc.tensor.matmul(y0[:], vT, e[:, 0:CH],
                                     start=(kt == 0), stop=(kt == TT - 1))
                    nc.tensor.matmul(y1[:], vT, e[:, CH:2 * CH],
                                     start=(kt == 0), stop=(kt == TT - 1))
                    # denominator tree: bf16 pairwise adds on DVE
                    if kt % 2 == 1:
                        t1 = spool.tile([P, 2 * CH], BF16, tag="t")
                        nc.vector.tensor_add(t1[:], es.pop(kt - 1)[:],
                                             es.pop(kt)[:])
                        lvl[1, kt // 2] = t1
                    for L in (1, 2, 3):
                        j = (kt + 1) // (1 << (L + 1))
                        if (kt + 1) % (1 << (L + 1)) == 0:
                            t2 = spool.tile([P, 2 * CH], BF16, tag="t")
                            nc.vector.tensor_add(
                                t2[:], lvl.pop((L, 2 * j - 2))[:],
                                lvl.pop((L, 2 * j - 1))[:])
                            lvl[L + 1, j - 1] = t2
                    yield
                sfin = lvl.pop((4, 0))
                # free the y psum banks early so the next pair's AV
                # accumulation never waits on this pair's recip/mul;
                # post-k these copies ride the Scalar queue (DVE is the
                # attention phase's tightest engine)
                yc0 = ycp.tile([P, CH], BF16, tag="yc", name="yc0")
                yc1 = ycp.tile([P, CH], BF16, tag="yc", name="yc1")
                if pi < 2:
                    nc.vector.tensor_copy(yc0[:], y0[:])
                    nc.vector.tensor_copy(yc1[:], y1[:])
                else:
                    nc.scalar.copy(yc0[:], y0[:])
                    nc.scalar.copy(yc1[:], y1[:])
                pend.append((h, cp, sfin, yc0, yc1))

            def all_pairs():
                for pi in range(NP):
                    yield from pair_gen(pi)

            pump = all_pairs()
            pumped = [0]

            def pump_units(n):
                for _ in range(n):
                    if next(pump, StopIteration) is StopIteration:
                        return False
                    pumped[0] += 1
                return True

            # ---------------- Phase K (+ injected pairs 0-1) ----------------
            # From here on the Scalar queue belongs to the exp stream:
            # every DMA trigger costs ~0.65us of issuing-engine time and a
            # WAR-waiting trigger blocks the whole queue, so all further
            # loads/stores trigger from Sync only.
            started = [False]
            for ci in range(NCH):
                if ci == 1:
                    msk[2] = load_masks(2, sync_only=True)
                    xq[2] = load_x(2, sync_only=True)
                elif ci == 2:
                    msk[3] = load_masks(3, sync_only=True)
                    xq[3] = load_x(3, sync_only=True)
                cs, sn = msk[ci]
                for fi in range(HPC):
                    ps = ps_k.tile([P, CH], F32, tag="mmk")
                    for kt in range(KT):
                        nc.tensor.matmul(
                            ps[:],
                            wk_t[fi // 4][kt][:, ts(fi % 4, P)],
                            xq[ci][kt][:],
                            start=(kt == 0),
                            stop=(kt == KT - 1),
                        )
                    k_t[fi, ci] = rope(ps, f"k{fi}_{ci}", cs, sn)
                    # pump early-pair units once their k chunks exist:
                    # pair-0 unit kt needs k_t[0, (kt+2)//4] for its s_mm
                    if fi >= 1:
                        if not started[0] and ci == 0:
                            s_store[0][0] = s_mm(0, 0)
                            s_store[0][1] = s_mm(0, 1)
                            started[0] = True
                        # pair-0 unit kt emits s_mm(kt+2) needing k chunk
                        # (kt+2)//4; pair 1 (h1) consumes the same chunks.
                        # Cap at 24 so pair-1's tail lands in attention.
                        max_units = (4 * (ci + 1) - 2) if ci < 3 else 24
                        per_win = 2 if ci < 3 else 3
                        pump_units(min(per_win, max_units - pumped[0]))
            ps_k.release()
            es1.close()

            # ---------------- attention (remaining pairs) ----------------
            ynp = outer.enter_context(tc.tile_pool(name="ynorm", bufs=1))
            wpp = outer.enter_context(tc.tile_pool(name="wp", bufs=1))
            rcp = outer.enter_context(
                tc.tile_pool(name="rc", bufs=2, side="right"))
            pap = outer.enter_context(
                tc.tile_pool(name="pa", bufs=6, side="right"))
            ep2 = outer.enter_context(
                tc.tile_pool(name="ee2", bufs=7, side="right"))
            ep2_pool[0] = ep2
            psD = tc.alloc_tile_pool(name="psD", bufs=2, space="PSUM")
            psd_pool[0] = psD
            rcp_pool[0] = rcp
            pap_pool[0] = pap
            for h in range(HPC):
                for half in range(2):
                    ynorm[h, half] = ynp.tile(
                        [P, 2 * CH], BF16, tag=f"yn{h}_{half}",
                        name=f"ynorm{h}_{half}")
            for h in range(HPC):
                wtl = wpp.tile([P, C], BF16, tag=f"wp{h}", name=f"wpt{h}")
                nc.sync.dma_start(wtl[:], wpd[h])
                wpt.append(wtl)

            while pump_units(16):
                pass

            # ---------------- Phase 3: boundary + remaining out-proj ----
            # flush the last tails with the reserved fill groups covering
            # the recip/mul chain, then hand the freed PSUM banks to the
            # final p3 tranche (ti 8-15, which needs the last tail).
            boundary[0] = 1
            while fillq:
                emit_fill()
            while pend:
                emit_tail(pend.popleft())
            psD.release()
            psY.release()
            psS.release()
            ps3 = tc.alloc_tile_pool(name="ps3", bufs=4, space="PSUM")
            while emit_fill(pool=ps3):
                pass
            ps3.release()

    nc.compile()
    return nc


def get_nc():
    global _CACHED_NC
    if _CACHED_NC is None:
        _CACHED_NC = build_nc()
    return _CACHED_NC


def make_rope_masks():
    half = D // 2
    inv = 1.0 / (ROPE_BASE ** (np.arange(half, dtype=np.float64) * 2.0 / D))
    ang = np.arange(T, dtype=np.float64)[:, None] * inv[None, :]  # [T, half]
    cos = np.cos(ang).T.astype(np.float32)  # [half, T]
    sin = np.sin(ang).T.astype(np.float32)
    cosm = np.empty((P, T), np.float32)
    sinm = np.empty((P, T), np.float32)
    cosm[0::2] = cos
    cosm[1::2] = cos
    sinm[0::2] = -sin
    sinm[1::2] = sin
    return cosm, sinm


def make_in_maps(x, w_attn, w_proj):
    import ml_dtypes
    BF = ml_dtypes.bfloat16

    x = np.asarray(x, dtype=np.float32)
    w_attn = np.asarray(w_attn, dtype=np.float32)
    w_proj = np.asarray(w_proj, dtype=np.float32)
    cosm, sinm = make_rope_masks()
    cosm16 = cosm.astype(BF)
    sinm16 = sinm.astype(BF)
    ones16 = np.ones((P, P), BF)
    in_maps = []
    for core in range(8):
        b, hg = core // 2, core % 2
        h0 = hg * HPC
        rq = slice(h0 * D, (h0 + HPC) * D)
        rk = slice(C + h0 * D, C + (h0 + HPC) * D)
        rv = slice(2 * C + h0 * D, 2 * C + (h0 + HPC) * D)
        # x tiles: [NCH, KT, P, CH] from x[b].T
        xt = np.ascontiguousarray(x[b].T.astype(BF))
        xtp = np.ascontiguousarray(
            xt.reshape(KT, P, NCH, CH).transpose(2, 0, 1, 3))
        # wq/wk/wv: [C, HPC*D] -> [KT, 2, P, CH] (1KB-row half tiles)
        def wtile(w):
            wT = w.T.astype(BF)  # [C, HPC*D]
            return np.ascontiguousarray(
                wT.reshape(KT, P, 2, CH).transpose(0, 2, 1, 3))
        wvd = wtile(w_attn[rv])
        wpT = np.ascontiguousarray(
            w_proj[:, h0 * D:(h0 + HPC) * D].T.astype(BF)).reshape(HPC, P, C)
        in_maps.append({
            "xtp": xtp,
            "wqd": wtile(w_attn[rq]),
            "wkd": wtile(w_attn[rk]),
            "wvd": wvd,
            "wpd": wpT,
            "cosm": cosm16,
            "sinm": sinm16,
            "onesd": ones16,
        })
    return in_maps


def combine_outputs(results):
    B = 4
    out = np.empty((B, T, C), np.float32)
    for b in range(B):
        out[b] = (results[2 * b]["out"].astype(np.float32)
                  + results[2 * b]["outa"].astype(np.float32)
                  + results[2 * b + 1]["out"].astype(np.float32)
                  + results[2 * b + 1]["outa"].astype(np.float32))
    return out


def kernel(x, w_attn, w_proj):
    from concourse.bass_utils import run_bass_kernel_spmd

    nc = get_nc()
    in_maps = make_in_maps(x, w_attn, w_proj)
    res = run_bass_kernel_spmd(nc, in_maps, list(range(8)))
    return combine_outputs(res.results)
